# revision 60
# baseline (speedup 1.0000x reference)
"""Trainium2 Bass kernel for the Wasserstein-attention transformer block.

Strategy: data-parallel over batch B=8 across 8 NeuronCores (one batch
element per core). Per core, the whole block runs with activations kept
in a transposed [feature, token] layout so every GEMM contracts over
partitions without runtime transposes of large tensors; attention runs
in S_T = [key, query] layout so softmax denominators and context
accumulation are plain matmuls. Matmul operands are bf16 (PSUM
accumulation fp32); the Wasserstein affine terms use f32r.

Host<->device traffic (the axon tunnel is ~35-65 MB/s and dominates the
wall clock) is minimized:
 - all shared tensors (weights + exp(rel_pos_bias)) are sent SHARDED 1/8
   per core and AllGather-ed on-device over the NeuronLink fabric;
 - weights and exp(rpb) travel as int4 (two nibbles per byte) with
   per-row f32 scales (gamma factors folded into the scales);
 - x inputs travel as int4 with per-token scales; the device returns
   only delta = y - x as per-token int4, and the host reconstructs
   y = x_f32 + dequant(delta), so the residual-stream precision never
   depends on the wire precision;
 - dispatch goes through a cached jit (no per-call retrace), donated
   output buffers are recycled from the previous call, and D2H pulls
   all shards concurrently.
"""
import contextlib
from concurrent.futures import ThreadPoolExecutor

import numpy as np
import ml_dtypes

import jax
import jax.numpy as jnp
from jax.experimental.shard_map import shard_map
from jax.sharding import Mesh, NamedSharding, PartitionSpec

import concourse.bass as bass
import concourse.tile as tile
from concourse import bacc, bass2jax, mybir
from concourse.masks import make_identity

F32 = mybir.dt.float32
F32R = mybir.dt.float32r
BF16 = mybir.dt.bfloat16
I8 = mybir.dt.int8
U8 = mybir.dt.uint8
F8 = mybir.dt.float8e4
AF = mybir.ActivationFunctionType
ALU = mybir.AluOpType
AXL = mybir.AxisListType

B, N, D, H = 8, 577, 768, 12
HD = D // H
DFF = 4 * D
SCALE = HD ** -0.5
LN_EPS = 1e-5

P = 128
NT = [(0, 128), (128, 128), (256, 128), (384, 128), (512, 65)]   # token tiles
QCH = [(0, 290), (290, 287)]                                     # psum-free chunks of N (both f32r-fast)
DT = D // P        # 6
FT = DFF // P      # 24
VCH = [(0, 384), (384, 384)]                                     # v / proj / fc2 out chunks

NC = 8             # cores
DSH = D // NC      # 96   row-shard of [D, *] weights
FSH = DFF // NC    # 384  row-shard of [DFF, *] weights
RPB_ROWS = H * N   # 6924
RPB_PAD = ((RPB_ROWS + NC - 1) // NC) * NC   # 6928
RPB_SH = RPB_PAD // NC                       # 866

GROUPS = [list(range(NC))]

_CACHE = {}


def _build_program():
    nc = bacc.Bacc("TRN2", target_bir_lowering=False, debug=False, num_devices=NC)

    # ---- DRAM I/O ----
    # per-core unique: int4 per-token-quantized x packed two-per-byte
    # (even col low nibble), scales in xs (col0=m, col1=c)
    xm_d = nc.declare_dram_parameter("xm", [N, D // 2], U8, isOutput=False)
    xc_d = nc.declare_dram_parameter("xc", [N, D // 2], U8, isOutput=False)
    xs_d = nc.declare_dram_parameter("xs", [N, 2], F32, isOutput=False)
    # sharded 1/8 per core, AllGather-ed on device; weights and exp(rpb)
    # travel as int4 packed two-per-byte with per-row f32 scales
    wqk_sh_d = nc.declare_dram_parameter("wqk_sh", [DSH, D], U8, isOutput=False)
    wv_sh_d = nc.declare_dram_parameter("wv_sh", [DSH, D // 2], U8, isOutput=False)
    rpb_sh_d = nc.declare_dram_parameter("rpb_sh", [RPB_SH, (N + 1) // 2], U8, isOutput=False)
    rpbs_sh_d = nc.declare_dram_parameter("rpbs_sh", [RPB_SH, 1], F32, isOutput=False)
    wpm_sh_d = nc.declare_dram_parameter("wpm_sh", [DSH, D // 2], U8, isOutput=False)
    wpc_sh_d = nc.declare_dram_parameter("wpc_sh", [DSH, D // 2], U8, isOutput=False)
    wfc1_sh_d = nc.declare_dram_parameter("wfc1_sh", [DSH, DFF // 2], U8, isOutput=False)
    wfc2_sh_d = nc.declare_dram_parameter("wfc2_sh", [FSH, D // 2], U8, isOutput=False)
    # small replicated f32: smA = qkbm(12) | qkbc(12) | fc1b(24) | wscales(54)
    # smB rows: vb, r1m, r1c, r2
    smA_d = nc.declare_dram_parameter("smA", [P, 102], F32, isOutput=False)
    smB_d = nc.declare_dram_parameter("smB", [4, D], F32, isOutput=False)
    # outputs: int4 per-token-quantized delta (y - x), two nibbles per byte
    # (even col in low nibble, odd in high); y rows 0..N-1 = mean stream,
    # N..2N-1 = cov stream; scales in ysc
    y_d = nc.declare_dram_parameter("y", [2 * N, D // 2], U8, isOutput=True)
    ysc_d = nc.declare_dram_parameter("ysc", [N, 2], F32, isOutput=True)

    with tile.TileContext(nc) as tc, contextlib.ExitStack() as top:
        # ---- on-device AllGather of the sharded shared tensors ----
        dram = top.enter_context(tc.tile_pool(name="dram_cc", bufs=1, space="DRAM"))

        def gathered(param, chunk, full, tag, dt=BF16):
            bt = dram.tile(list(chunk), dt, tag=f"b_{tag}", name=f"b_{tag}")
            gt = dram.tile(list(full), dt, tag=f"g_{tag}", name=f"g_{tag}")
            nc.sync.dma_start(out=bt[:], in_=param[:])
            nc.gpsimd.collective_compute(
                "AllGather", ALU.bypass, replica_groups=GROUPS,
                ins=[bt.opt()], outs=[gt.opt()])
            return gt

        # ordered by first use: qkv -> rpb -> proj -> mlp
        NP2 = (N + 1) // 2
        wqk_g = gathered(wqk_sh_d, (DSH, D), (D, D), "wqk", dt=U8)
        wv_g = gathered(wv_sh_d, (DSH, D // 2), (D, D // 2), "wv", dt=U8)
        rpb_g = gathered(rpb_sh_d, (RPB_SH, NP2), (RPB_PAD, NP2), "rpb", dt=U8)
        rpbs_g = gathered(rpbs_sh_d, (RPB_SH, 1), (RPB_PAD, 1), "rpbs", dt=F32)
        wpm_g = gathered(wpm_sh_d, (DSH, D // 2), (D, D // 2), "wpm", dt=U8)
        wpc_g = gathered(wpc_sh_d, (DSH, D // 2), (D, D // 2), "wpc", dt=U8)
        wfc1_g = gathered(wfc1_sh_d, (DSH, DFF // 2), (D, DFF // 2), "wfc1", dt=U8)
        wfc2_g = gathered(wfc2_sh_d, (FSH, D // 2), (DFF, D // 2), "wfc2", dt=U8)

        const = top.enter_context(tc.tile_pool(name="const", bufs=1))
        persist = top.enter_context(tc.tile_pool(name="persist", bufs=1))

        ident = const.tile([P, P], BF16, tag="ident", name="ident")
        make_identity(nc, ident)
        eps_t = const.tile([P, 1], F32, tag="eps", name="eps")
        nc.vector.memset(eps_t, LN_EPS)
        half_t = const.tile([P, 1], F32, tag="half", name="half")
        nc.vector.memset(half_t, 0.5)
        negh_f = const.tile([P, 2], F32, tag="negh_f", name="negh_f")
        nc.vector.memset(negh_f, -0.5)
        negh = const.tile([P, 2], F32R, tag="negh", name="negh")
        nc.vector.tensor_copy(out=negh[:], in_=negh_f[:])
        ones_f = const.tile([1, N], F32, tag="ones_f", name="ones_f")
        nc.vector.memset(ones_f, 1.0)
        ones_r = const.tile([1, N], F32R, tag="ones_r", name="ones_r")
        nc.vector.tensor_copy(out=ones_r[:], in_=ones_f[:])

        # biases / rows / weight scales, packed in smA / smB
        smA = persist.tile([P, 102], F32, tag="smA", name="smA")
        nc.sync.dma_start(out=smA[:], in_=smA_d[:])
        # smA columns: qkbm 0-11 | qkbc 12-23 | fc1b 24-47 | wscales 48-101
        # (wscales: wqk +0..5 | wv +6..11 | wpm +12..17 | wpc +18..23 |
        #  wfc1 +24..29 | wfc2 +30..53)
        vb_b = persist.tile([P, D], F32, tag="vb_b", name="vb_b")
        nc.sync.dma_start(out=vb_b[:], in_=smB_d[0:1, :].to_broadcast([P, D]))
        r1m_b = persist.tile([P, D], F32, tag="r1m_b", name="r1m_b")
        nc.sync.dma_start(out=r1m_b[:], in_=smB_d[1:2, :].to_broadcast([P, D]))
        r1c_b = persist.tile([P, D], F32, tag="r1c_b", name="r1c_b")
        nc.sync.dma_start(out=r1c_b[:], in_=smB_d[2:3, :].to_broadcast([P, D]))
        r2_b = persist.tile([P, D], F32, tag="r2_b", name="r2_b")
        nc.sync.dma_start(out=r2_b[:], in_=smB_d[3:4, :].to_broadcast([P, D]))

        # int4 -> bf16 weight loads: unpack nibbles, scale per in-feature row
        def load_w4(pool, pw, dst, src_ap, scale_ap, wp):
            st = pool.tile([P, pw], U8, tag="wst", name="wst")
            lo = pool.tile([P, pw], U8, tag="wlo", name="wlo")
            hi = pool.tile([P, pw], U8, tag="whi", name="whi")
            nc.sync.dma_start(out=st[:, :wp], in_=src_ap)
            nc.vector.tensor_scalar(out=lo[:, :wp], in0=st[:, :wp], scalar1=15,
                                    scalar2=None, op0=ALU.bitwise_and)
            nc.vector.tensor_scalar(out=hi[:, :wp], in0=st[:, :wp], scalar1=4,
                                    scalar2=None, op0=ALU.logical_shift_right)
            dg = dst.rearrange("p (d t) -> p t d", t=2)
            nc.vector.tensor_scalar(out=dg[:, 0, :], in0=lo[:, :wp], scalar1=8.0,
                                    scalar2=scale_ap, op0=ALU.subtract, op1=ALU.mult)
            nc.vector.tensor_scalar(out=dg[:, 1, :], in0=hi[:, :wp], scalar1=8.0,
                                    scalar2=scale_ap, op0=ALU.subtract, op1=ALU.mult)

        # residual-stream tiles (bf16, natural layout); become x' in place.
        # Loaded as int8 + per-token scale; delta tiles d_t accumulate the
        # gamma-scaled branch sums (the device output is delta = y - x).
        xs_t = []
        for i, (n0, nn) in enumerate(NT):
            t = persist.tile([P, 2], F32, tag=f"xs{i}", name=f"xs{i}")
            nc.sync.dma_start(out=t[:nn, :], in_=xs_d[n0:n0 + nn, :])
            xs_t.append(t)
        x_t, d_t = {}, {}
        xqp = top.enter_context(tc.tile_pool(name="xq", bufs=3))
        for si, (s, src) in enumerate((("m", xm_d), ("c", xc_d))):
            for i, (n0, nn) in enumerate(NT):
                xq = xqp.tile([P, D // 2], U8, tag="xq", name="xq")
                nc.sync.dma_start(out=xq[:nn, :], in_=src[n0:n0 + nn, :])
                lo = xqp.tile([P, D // 2], U8, tag="xlo", name="xlo")
                hi = xqp.tile([P, D // 2], U8, tag="xhi", name="xhi")
                nc.vector.tensor_scalar(out=lo[:nn, :], in0=xq[:nn, :], scalar1=15,
                                        scalar2=None, op0=ALU.bitwise_and)
                nc.vector.tensor_scalar(out=hi[:nn, :], in0=xq[:nn, :], scalar1=4,
                                        scalar2=None, op0=ALU.logical_shift_right)
                t = persist.tile([P, D], BF16, tag=f"x_{s}{i}", name=f"x_{s}{i}")
                tg = t[:nn, :].rearrange("p (d t) -> p t d", t=2)
                sc_ap = xs_t[i][:nn, si:si + 1]
                nc.vector.tensor_scalar(out=tg[:, 0, :], in0=lo[:nn, :], scalar1=8.0,
                                        scalar2=sc_ap, op0=ALU.subtract, op1=ALU.mult)
                nc.vector.tensor_scalar(out=tg[:, 1, :], in0=hi[:nn, :], scalar1=8.0,
                                        scalar2=sc_ap, op0=ALU.subtract, op1=ALU.mult)
                x_t[s, i] = t
                d_t[s, i] = persist.tile([P, D], BF16, tag=f"d_{s}{i}", name=f"d_{s}{i}")

        # ---------- helpers ----------
        def layernorm_transpose(lnp, psln, s, xhatT):
            """LN over feature dim of x_t[s,*] then transpose into xhatT[j] tiles."""
            for i, (n0, nn) in enumerate(NT):
                xt = x_t[s, i]
                stats = lnp.tile([P, 3, 6], F32, tag="stats", name="stats")
                xg = xt[:nn, :].rearrange("p (g d) -> p g d", g=3)
                for g in range(3):
                    nc.vector.bn_stats(out=stats[:nn, g, :], in_=xg[:, g, :])
                mv = lnp.tile([P, 2], F32, tag="mv", name="mv")
                nc.vector.bn_aggr(out=mv[:nn], in_=stats[:nn])
                rstd = lnp.tile([P, 1], F32, tag="rstd", name="rstd")
                nc.scalar.activation(out=rstd[:nn], in_=mv[:nn, 1:2], func=AF.Sqrt,
                                     bias=eps_t[:nn], scale=1.0)
                nc.vector.reciprocal(out=rstd[:nn], in_=rstd[:nn])
                xhat = lnp.tile([P, D], BF16, tag="xhat", name="xhat")
                nc.vector.tensor_scalar(out=xhat[:nn], in0=xt[:nn, :],
                                        scalar1=mv[:nn, 0:1], scalar2=rstd[:nn],
                                        op0=ALU.subtract, op1=ALU.mult)
                for j in range(DT):
                    pst = psln.tile([P, P], BF16, tag="pst", name="pst")
                    nc.tensor.transpose(out=pst[:, :nn], in_=xhat[:nn, j * P:(j + 1) * P],
                                        identity=ident[:nn, :nn])
                    if j % 2 == 0:
                        nc.scalar.copy(out=xhatT[j][:, n0:n0 + nn], in_=pst[:, :nn])
                    else:
                        nc.vector.tensor_copy(out=xhatT[j][:, n0:n0 + nn], in_=pst[:, :nn])

        # ================= Phase A/B: LN1 + QKV =================
        # Pool lifetimes are a stack (LIFO release): ctx_io spans A/B..D and is
        # opened first; attn_io spans A/B..C and closes right after attention.
        ctx_cm = tc.tile_pool(name="ctx_io", bufs=1)
        ctx_io = ctx_cm.__enter__()
        ctxm = [ctx_io.tile([P, N], BF16, tag=f"ctxm{j}", name=f"ctxm{j}") for j in range(DT)]
        ctxc = [ctx_io.tile([P, N], BF16, tag=f"ctxc{j}", name=f"ctxc{j}") for j in range(DT)]
        attn_cm = tc.tile_pool(name="attn_io", bufs=1)
        attn_io = attn_cm.__enter__()
        qc = [attn_io.tile([P, N], BF16, tag=f"qc{h}", name=f"qc{h}") for h in range(H)]
        kc = [attn_io.tile([P, N], BF16, tag=f"kc{h}", name=f"kc{h}") for h in range(H)]
        vm = [attn_io.tile([P, H, HD + 1], BF16, tag=f"vm{i}", name=f"vm{i}") for i in range(5)]
        vc = [attn_io.tile([P, H, HD], BF16, tag=f"vc{i}", name=f"vc{i}") for i in range(5)]
        for i, (n0, nn) in enumerate(NT):
            nc.vector.memset(vm[i][:nn, :, HD:HD + 1], 1.0)

        with contextlib.ExitStack() as ab:
            wpool = ab.enter_context(tc.tile_pool(name="wqkv", bufs=1))
            wqk = [wpool.tile([P, 2 * D], BF16, tag=f"wqk{j}", name=f"wqk{j}") for j in range(DT)]
            wv = [wpool.tile([P, D], BF16, tag=f"wv{j}", name=f"wv{j}") for j in range(DT)]
            wstg1 = ab.enter_context(tc.tile_pool(name="wstg1", bufs=2))
            for j in range(DT):
                load_w4(wstg1, D, wqk[j][:], wqk_g[j * P:(j + 1) * P, :],
                        smA[:, 48 + j:49 + j], D)
                load_w4(wstg1, D, wv[j][:], wv_g[j * P:(j + 1) * P, :],
                        smA[:, 54 + j:55 + j], D // 2)

            xhatT = {s: [wpool.tile([P, N], BF16, tag=f"xhatT_{s}{j}", name=f"xhatT_{s}{j}") for j in range(DT)]
                     for s in ("m", "c")}
            lnp1 = ab.enter_context(tc.tile_pool(name="ln_ln1", bufs=3))
            psln1 = ab.enter_context(tc.tile_pool(name="psln_ln1", bufs=2, space="PSUM"))
            for s in ("m", "c"):
                layernorm_transpose(lnp1, psln1, s, xhatT[s])

            psqk = ab.enter_context(tc.tile_pool(name="psqk", bufs=3, space="PSUM"))
            sc1 = ab.enter_context(tc.tile_pool(name="sc_covqk", bufs=3))

            # --- QK GEMMs, transposed layout out [d_out, n] ---
            for s in ("m", "c"):
                for t in range(2 * DT):           # 6 q-tiles then 6 k-tiles
                    is_q = t < DT
                    for (c0, cw) in QCH:
                        ps = psqk.tile([P, 512], F32, tag="ps", name="ps")
                        for j in range(DT):
                            nc.tensor.matmul(ps[:, :cw], lhsT=wqk[j][:, t * P:(t + 1) * P],
                                             rhs=xhatT[s][j][:, c0:c0 + cw],
                                             start=(j == 0), stop=(j == DT - 1))
                        hpair = (t % DT) * 2      # heads 2*(t%6), +1
                        dst = qc if is_q else kc
                        if s == "m":
                            # mean stream: out = sc*(z + b)
                            sc = SCALE if is_q else 1.0
                            for half in range(2):
                                pr = slice(64 * half, 64 * half + 64)
                                nc.vector.tensor_scalar(
                                    out=dst[hpair + half][0:64, c0:c0 + cw],
                                    in0=ps[pr, :cw], scalar1=smA[pr, t:t + 1],
                                    scalar2=sc, op0=ALU.add, op1=ALU.mult)
                        else:
                            # cov stream: c = sqrt(elu(z + b) + 1)
                            t1 = sc1.tile([P, 512], F32, tag="t1", name="t1")
                            nc.vector.tensor_scalar_add(out=t1[:, :cw], in0=ps[:, :cw],
                                                        scalar1=smA[:, 12 + t:13 + t])
                            t2 = sc1.tile([P, 512], F32, tag="t2", name="t2")
                            nc.vector.tensor_scalar_min(out=t2[:, :cw], in0=t1[:, :cw], scalar1=0.0)
                            nc.scalar.activation(out=t2[:, :cw], in_=t2[:, :cw], func=AF.Exp)
                            nc.vector.scalar_tensor_tensor(out=t1[:, :cw], in0=t1[:, :cw],
                                                           scalar=0.0, in1=t2[:, :cw],
                                                           op0=ALU.max, op1=ALU.add)
                            for half in range(2):
                                pr = slice(64 * half, 64 * half + 64)
                                nc.scalar.activation(
                                    out=dst[hpair + half][64:128, c0:c0 + cw],
                                    in_=t1[pr, :cw], func=AF.Sqrt)

            # --- V GEMMs, natural layout out [n, d_v] ---
            for s in ("m", "c"):
                for i, (n0, nn) in enumerate(NT):
                    for c2, (v0, vw) in enumerate(VCH):
                        ps = psqk.tile([P, 512], F32, tag="ps", name="ps")
                        for j in range(DT):
                            nc.tensor.matmul(ps[:nn, :vw], lhsT=xhatT[s][j][:, n0:n0 + nn],
                                             rhs=wv[j][:, v0:v0 + vw],
                                             start=(j == 0), stop=(j == DT - 1))
                        psg = ps[:nn, :vw].rearrange("p (g d) -> p g d", g=6)
                        vbg = vb_b[:nn, v0:v0 + vw].rearrange("p (g d) -> p g d", g=6)
                        hs = slice(6 * c2, 6 * c2 + 6)
                        if s == "m":
                            nc.vector.tensor_tensor(out=vm[i][:nn, hs, 0:HD], in0=psg,
                                                    in1=vbg, op=ALU.add)
                        else:
                            t1 = sc1.tile([P, 512], F32, tag="t1", name="t1")
                            t1g = t1[:nn, :vw].rearrange("p (g d) -> p g d", g=6)
                            nc.vector.tensor_tensor(out=t1g, in0=psg, in1=vbg, op=ALU.add)
                            t2 = sc1.tile([P, 512], F32, tag="t2", name="t2")
                            nc.vector.tensor_scalar_min(out=t2[:nn, :vw], in0=t1[:nn, :vw],
                                                        scalar1=0.0)
                            nc.scalar.activation(out=t2[:nn, :vw], in_=t2[:nn, :vw], func=AF.Exp)
                            t2g = t2[:nn, :vw].rearrange("p (g d) -> p g d", g=6)
                            nc.vector.scalar_tensor_tensor(out=vc[i][:nn, hs, :], in0=t1g,
                                                           scalar=0.0, in1=t2g,
                                                           op0=ALU.max, op1=ALU.add)

        # ================= Phase C: attention =================
        with contextlib.ExitStack() as at:
            AB = at.enter_context(tc.tile_pool(name="AB", bufs=1))
            # per-head K=2 affine operands packed at 32-aligned partition slots
            # (base partition must be 0/32/64): head h -> tile h//3,
            # partitions (h%3)*32 + {0,1}. A = [colterm; ones], B = [ones; rowterm]
            N2 = N + 1   # fp32r needs even innermost extents; pad column never read
            A_pack = [AB.tile([P, N2], F32R, tag=f"A_pack{t}", name=f"A_pack{t}") for t in range(4)]
            B_pack = [AB.tile([P, N2], F32R, tag=f"B_pack{t}", name=f"B_pack{t}") for t in range(4)]

            def ab_slot(h):
                return A_pack[h // 3], B_pack[h // 3], (h % 3) * 32
            sqp = at.enter_context(tc.tile_pool(name="sqp", bufs=2))
            stg = at.enter_context(tc.tile_pool(name="stg", bufs=2))
            sigp = at.enter_context(tc.tile_pool(name="sigp", bufs=5))
            rpbp = at.enter_context(tc.tile_pool(name="rpbp", bufs=5))
            ep = at.enter_context(tc.tile_pool(name="ep", bufs=12))
            denp = at.enter_context(tc.tile_pool(name="denp", bufs=2))
            rcb = at.enter_context(tc.tile_pool(name="rcb", bufs=2))
            ps_r = at.enter_context(tc.tile_pool(name="ps_r", bufs=2, space="PSUM"))
            ps_s = at.enter_context(tc.tile_pool(name="ps_s", bufs=2, space="PSUM"))
            ps_c = at.enter_context(tc.tile_pool(name="ps_c", bufs=1, space="PSUM"))

            for h in range(H):
                # affine terms: A=[ -0.5*|w_k|^2 ; 1 ], B=[ 1 ; -0.5*|u_q|^2 ]
                A_t, B_t, sl = ab_slot(h)
                nc.sync.dma_start(out=A_t[sl + 1:sl + 2, :N], in_=ones_r[:])
                nc.vector.tensor_copy(out=B_t[sl:sl + 1, :N], in_=ones_r[:])
                sq_k = sqp.tile([P, N2], F32R, tag="sq", name="sq")
                nc.vector.tensor_tensor(out=sq_k[:, :N], in0=kc[h][:], in1=kc[h][:], op=ALU.mult)
                for (c0, cw) in QCH:
                    cwe = cw + (cw % 2)
                    pr = ps_r.tile([2, 512], F32, tag="pr", name="pr")
                    nc.tensor.matmul(pr[:, :cwe], lhsT=negh[:], rhs=sq_k[:, c0:c0 + cwe],
                                     start=True, stop=True)
                    nc.scalar.copy(out=A_t[sl:sl + 1, c0:c0 + cw], in_=pr[0:1, :cw])
                sq_q = sqp.tile([P, N2], F32R, tag="sq", name="sq")
                nc.vector.tensor_tensor(out=sq_q[:, :N], in0=qc[h][:], in1=qc[h][:], op=ALU.mult)
                rowst = stg.tile([1, N], F32R, tag="rowst", name="rowst")
                for (c0, cw) in QCH:
                    cwe = cw + (cw % 2)
                    pr = ps_r.tile([2, 512], F32, tag="pr", name="pr")
                    nc.tensor.matmul(pr[:, :cwe], lhsT=negh[:], rhs=sq_q[:, c0:c0 + cwe],
                                     start=True, stop=True)
                    nc.scalar.copy(out=rowst[0:1, c0:c0 + cw], in_=pr[0:1, :cw])
                nc.sync.dma_start(out=B_t[sl + 1:sl + 2, :N], in_=rowst[:])

                # scores + sigmoid + rpb + exp, S_T layout [k, q]
                e_h, e2_h = [], []
                for kt, (k0, kn) in enumerate(NT):
                    r0 = h * N + k0
                    rpq = rpbp.tile([P, NP2], U8, tag="rpb", name="rpb")
                    nc.sync.dma_start(out=rpq[:kn, :], in_=rpb_g[r0:r0 + kn, :])
                    rps = rpbp.tile([P, 1], F32, tag="rps", name="rps")
                    nc.sync.dma_start(out=rps[:kn, :], in_=rpbs_g[r0:r0 + kn, :])
                    rlo = rpbp.tile([P, NP2], U8, tag="rlo", name="rlo")
                    rhi = rpbp.tile([P, NP2], U8, tag="rhi", name="rhi")
                    nc.vector.tensor_scalar(out=rlo[:kn, :], in0=rpq[:kn, :], scalar1=15,
                                            scalar2=None, op0=ALU.bitwise_and)
                    nc.vector.tensor_scalar(out=rhi[:kn, :], in0=rpq[:kn, :], scalar1=4,
                                            scalar2=None, op0=ALU.logical_shift_right)
                    rpb_b = rpbp.tile([P, 2 * NP2], BF16, tag="rpbb", name="rpbb")
                    rg = rpb_b[:kn, :].rearrange("p (d t) -> p t d", t=2)
                    nc.vector.tensor_scalar_mul(out=rg[:, 0, :], in0=rlo[:kn, :],
                                                scalar1=rps[:kn, 0:1])
                    nc.vector.tensor_scalar_mul(out=rg[:, 1, :], in0=rhi[:kn, :],
                                                scalar1=rps[:kn, 0:1])
                    sig = sigp.tile([P, N], F32, tag="sig", name="sig")
                    e_t = ep.tile([P, N], BF16, tag="e", name="e")
                    e2_t = ep.tile([P, N], BF16, tag="e2", name="e2")
                    for (c0, cw) in QCH:
                        ps = ps_s.tile([P, 512], F32, tag="ps", name="ps")
                        A_t, B_t, sl = ab_slot(h)
                        kne = kn + (kn % 2)
                        cwe = cw + (cw % 2)
                        nc.tensor.matmul(ps[:kn, :cw], lhsT=kc[h][:, k0:k0 + kn],
                                         rhs=qc[h][:, c0:c0 + cw], start=True, stop=False)
                        nc.tensor.matmul(ps[:kne, :cwe], lhsT=A_t[sl:sl + 2, k0:k0 + kne],
                                         rhs=B_t[sl:sl + 2, c0:c0 + cwe], start=False, stop=True,
                                         skip_group_check=True)
                        # sigmoid(2x) = 0.5*tanh(x) + 0.5; tanh shares the ACT
                        # table set with exp.
                        nc.scalar.activation(out=sig[:kn, c0:c0 + cw], in_=ps[:kn, :cw],
                                             func=AF.Tanh, scale=1.0)
                    # e = exp(0.5*tanh + 0.5) * exp(rpb)   (rpb sent as fp8 factor)
                    nc.scalar.activation(out=sig[:kn, :], in_=sig[:kn, :], func=AF.Exp,
                                         bias=half_t[:kn], scale=0.5)
                    nc.vector.tensor_tensor(out=e_t[:kn, :], in0=sig[:kn, :],
                                            in1=rpb_b[:kn, :N], op=ALU.mult)
                    nc.gpsimd.tensor_tensor(out=e2_t[:kn, :], in0=e_t[:kn, :],
                                            in1=e_t[:kn, :], op=ALU.mult)
                    e_h.append(e_t)
                    e2_h.append(e2_t)

                # context matmuls (unnormalized) + per-chunk denominator:
                # each chunk's reciprocal/broadcast/evict chain depends only on
                # its own denominator slice, so chunks (and heads) pipeline.
                den = denp.tile([1, N], F32, tag="den", name="den")
                recip = denp.tile([1, N], F32, tag="recip", name="recip")
                rb = rcb.tile([64, N], F32, tag="rb", name="rb")
                rb2 = rcb.tile([64, N], F32, tag="rb2", name="rb2")
                jt, rr = h // 2, slice(64 * (h % 2), 64 * (h % 2) + 64)
                for ci, (c0, cw) in enumerate(QCH):
                    pm = ps_c.tile([65, 512], F32, tag=f"pcm{ci}", name=f"pcm{ci}")
                    pc2 = ps_c.tile([64, 512], F32, tag=f"pcc{ci}", name=f"pcc{ci}")
                    for kt, (k0, kn) in enumerate(NT):
                        nc.tensor.matmul(pm[:, :cw], lhsT=vm[kt][:kn, h, :],
                                         rhs=e_h[kt][:kn, c0:c0 + cw],
                                         start=(kt == 0), stop=(kt == 4))
                        nc.tensor.matmul(pc2[:, :cw], lhsT=vc[kt][:kn, h, :],
                                         rhs=e2_h[kt][:kn, c0:c0 + cw],
                                         start=(kt == 0), stop=(kt == 4))
                    nc.scalar.copy(out=den[0:1, c0:c0 + cw], in_=pm[64:65, :cw])
                    nc.vector.reciprocal(out=recip[0:1, c0:c0 + cw],
                                         in_=den[0:1, c0:c0 + cw])
                    nc.gpsimd.partition_broadcast(rb[:, c0:c0 + cw],
                                                  recip[0:1, c0:c0 + cw])
                    nc.vector.tensor_tensor(out=rb2[:, c0:c0 + cw],
                                            in0=rb[:, c0:c0 + cw],
                                            in1=rb[:, c0:c0 + cw], op=ALU.mult)
                    nc.vector.tensor_tensor(out=ctxm[jt][rr, c0:c0 + cw],
                                            in0=pm[0:64, :cw],
                                            in1=rb[:, c0:c0 + cw], op=ALU.mult)
                    nc.vector.tensor_tensor(out=ctxc[jt][rr, c0:c0 + cw],
                                            in0=pc2[0:64, :cw],
                                            in1=rb2[:, c0:c0 + cw], op=ALU.mult)

        attn_cm.__exit__(None, None, None)

        # ================= Phase D: proj + residual =================
        with contextlib.ExitStack() as pd:
            wpp = pd.enter_context(tc.tile_pool(name="wproj", bufs=1))
            wpm = [wpp.tile([P, D], BF16, tag=f"wpm{j}", name=f"wpm{j}") for j in range(DT)]
            wpc = [wpp.tile([P, D], BF16, tag=f"wpc{j}", name=f"wpc{j}") for j in range(DT)]
            wstg2 = pd.enter_context(tc.tile_pool(name="wstg2", bufs=2))
            for j in range(DT):
                load_w4(wstg2, D // 2, wpm[j][:], wpm_g[j * P:(j + 1) * P, :],
                        smA[:, 60 + j:61 + j], D // 2)
                load_w4(wstg2, D // 2, wpc[j][:], wpc_g[j * P:(j + 1) * P, :],
                        smA[:, 66 + j:67 + j], D // 2)
            psp = pd.enter_context(tc.tile_pool(name="psproj", bufs=3, space="PSUM"))
            for s, ctx_t, wp, rb_row in (("m", ctxm, wpm, r1m_b), ("c", ctxc, wpc, r1c_b)):
                for i, (n0, nn) in enumerate(NT):
                    for (v0, vw) in VCH:
                        ps = psp.tile([P, 512], F32, tag="ps", name="ps")
                        for j in range(DT):
                            nc.tensor.matmul(ps[:nn, :vw], lhsT=ctx_t[j][:, n0:n0 + nn],
                                             rhs=wp[j][:, v0:v0 + vw],
                                             start=(j == 0), stop=(j == DT - 1))
                        xt, dt = x_t[s, i], d_t[s, i]
                        # dd = gamma1*proj(ctx) + r1  (gamma folded into w-scales)
                        nc.vector.tensor_tensor(out=dt[:nn, v0:v0 + vw],
                                                in0=ps[:nn, :vw],
                                                in1=rb_row[:nn, v0:v0 + vw], op=ALU.add)
                        nc.vector.tensor_tensor(out=xt[:nn, v0:v0 + vw],
                                                in0=xt[:nn, v0:v0 + vw],
                                                in1=dt[:nn, v0:v0 + vw], op=ALU.add)

        ctx_cm.__exit__(None, None, None)

        # ================= Phase E/F: LN2 + MLP =================
        with contextlib.ExitStack() as pf:
            wfp = pf.enter_context(tc.tile_pool(name="wfc", bufs=1))
            wfc1 = [wfp.tile([P, DFF], BF16, tag=f"wfc1_{j}", name=f"wfc1_{j}") for j in range(DT)]
            wstg3 = pf.enter_context(tc.tile_pool(name="wstg3", bufs=2))
            for j in range(DT):
                load_w4(wstg3, DFF // 2, wfc1[j][:], wfc1_g[j * P:(j + 1) * P, :],
                        smA[:, 72 + j:73 + j], DFF // 2)
            wfc2 = [wfp.tile([P, D], BF16, tag=f"wfc2_{f}", name=f"wfc2_{f}") for f in range(FT)]
            for f in range(FT):
                load_w4(wstg3, DFF // 2, wfc2[f][:], wfc2_g[f * P:(f + 1) * P, :],
                        smA[:, 78 + f:79 + f], D // 2)

            xhat2T = {s: [wfp.tile([P, N], BF16, tag=f"xh2T_{s}{j}", name=f"xh2T_{s}{j}") for j in range(DT)]
                      for s in ("m", "c")}
            lnp2 = pf.enter_context(tc.tile_pool(name="ln_ln2", bufs=3))
            psln2 = pf.enter_context(tc.tile_pool(name="psln_ln2", bufs=2, space="PSUM"))
            for s in ("m", "c"):
                layernorm_transpose(lnp2, psln2, s, xhat2T[s])

            psf = pf.enter_context(tc.tile_pool(name="psfc", bufs=4, space="PSUM"))
            hp = pf.enter_context(tc.tile_pool(name="hT", bufs=1))
            outp = pf.enter_context(tc.tile_pool(name="outp", bufs=3))
            for si, s in enumerate(("m", "c")):
                # hT tiles shared between streams (tag reuse serializes via deps)
                hT = {s: [hp.tile([P, N], BF16, tag=f"hT{f}", name=f"hT{f}")
                          for f in range(FT)]}
                for f in range(FT):
                    for (c0, cw) in QCH:
                        ps = psf.tile([P, 512], F32, tag="ps", name="ps")
                        for j in range(DT):
                            nc.tensor.matmul(ps[:, :cw], lhsT=wfc1[j][:, f * P:(f + 1) * P],
                                             rhs=xhat2T[s][j][:, c0:c0 + cw],
                                             start=(j == 0), stop=(j == DT - 1))
                        nc.scalar.activation(out=hT[s][f][:, c0:c0 + cw], in_=ps[:, :cw],
                                             func=AF.Gelu, bias=smA[:, 24 + f:25 + f],
                                             scale=1.0)
                for i, (n0, nn) in enumerate(NT):
                    # delta = d_attn + gamma2*mlp(...) + r2; quantize per token
                    yt = outp.tile([P, D], F32, tag="yt", name="yt")
                    yq = outp.tile([P, D], U8, tag="yq", name="yq")
                    for (v0, vw) in VCH:
                        ps = psf.tile([P, 512], F32, tag="ps", name="ps")
                        for f in range(FT):
                            nc.tensor.matmul(ps[:nn, :vw], lhsT=hT[s][f][:, n0:n0 + nn],
                                             rhs=wfc2[f][:, v0:v0 + vw],
                                             start=(f == 0), stop=(f == FT - 1))
                        # gamma2*fc2(h) + d_attn  (gamma folded into w-scales)
                        nc.vector.tensor_tensor(out=yt[:nn, v0:v0 + vw], in0=ps[:nn, :vw],
                                                in1=d_t[s, i][:nn, v0:v0 + vw], op=ALU.add)
                        nc.vector.tensor_tensor(out=yt[:nn, v0:v0 + vw],
                                                in0=yt[:nn, v0:v0 + vw],
                                                in1=r2_b[:nn, v0:v0 + vw], op=ALU.add)
                    am = outp.tile([P, 1], F32, tag="am", name="am")
                    nc.vector.tensor_reduce(out=am[:nn], in_=yt[:nn, :], axis=AXL.X,
                                            op=ALU.max, apply_absolute_value=True)
                    sc_t = outp.tile([P, 1], F32, tag="sc", name="sc")
                    nc.vector.tensor_scalar(out=sc_t[:nn], in0=am[:nn],
                                            scalar1=1.0 / 7.0, scalar2=1e-30,
                                            op0=ALU.mult, op1=ALU.max)
                    inv = outp.tile([P, 1], F32, tag="inv", name="inv")
                    nc.vector.reciprocal(out=inv[:nn], in_=sc_t[:nn])
                    # biased nibble q+8 in [1,15], then pack pairs: lo|hi<<4
                    nc.vector.tensor_scalar(out=yq[:nn, :], in0=yt[:nn, :],
                                            scalar1=inv[:nn], scalar2=8.0,
                                            op0=ALU.mult, op1=ALU.add)
                    qg = yq[:nn, :].rearrange("p (d t) -> p t d", t=2)
                    pk = outp.tile([P, D // 2], U8, tag="pk", name="pk")
                    nc.vector.scalar_tensor_tensor(out=pk[:nn, :], in0=qg[:, 1, :],
                                                   scalar=16.0, in1=qg[:, 0, :],
                                                   op0=ALU.mult, op1=ALU.add)
                    nc.sync.dma_start(out=y_d[si * N + n0:si * N + n0 + nn, :],
                                      in_=pk[:nn, :])
                    nc.sync.dma_start(out=ysc_d[n0:n0 + nn, si:si + 1], in_=sc_t[:nn])

    nc.compile()
    return nc


def _prep_shared(inputs):
    f32 = np.float32
    g = lambda k: np.asarray(inputs[k], f32)
    qkv_w, norm1_w, norm1_b = g("qkv_w"), g("norm1_w"), g("norm1_b")
    qkv_w_eff = qkv_w * norm1_w[None, :]
    qkv_b_eff = qkv_w_eff @ norm1_b

    wqkT = np.ascontiguousarray(qkv_w_eff[:2 * D].T)
    wvT = np.ascontiguousarray(qkv_w_eff[2 * D:].T)
    qkb = qkv_b_eff[:2 * D]
    vb = qkv_b_eff[2 * D:]

    gamma1, gamma2 = g("gamma1"), g("gamma2")
    proj_w, proj_b = g("proj_w"), g("proj_b")
    cov_proj_w, cov_proj_b = g("cov_proj_w"), g("cov_proj_b")
    norm2_w, norm2_b = g("norm2_w"), g("norm2_b")
    fc1_w, fc1_b = g("fc1_w"), g("fc1_b")
    fc2_w, fc2_b = g("fc2_w"), g("fc2_b")

    fc1_w_eff = fc1_w * norm2_w[None, :]
    fc1_b_eff = fc1_b + fc1_w_eff @ norm2_b

    def pack_w(Wt):
        """Per-row symmetric int4, nibble-packed; returns (u8 [r, c/2], s [r])."""
        amax = np.abs(Wt).max(1)
        s = np.maximum(amax, 1e-30) / 7.0
        q8 = (np.rint(Wt / s[:, None]) + 8.0).astype(np.uint8)
        return q8[:, 0::2] | (q8[:, 1::2] << 4), s.astype(f32)

    p_qk, s_qk = pack_w(wqkT)
    p_v, s_v = pack_w(wvT)
    p_pm, s_pm = pack_w(np.ascontiguousarray((gamma1[:, None] * proj_w).T))
    p_pc, s_pc = pack_w(np.ascontiguousarray((gamma1[:, None] * cov_proj_w).T))
    p_f1, s_f1 = pack_w(np.ascontiguousarray(fc1_w_eff.T))
    p_f2, s_f2 = pack_w(np.ascontiguousarray((gamma2[:, None] * fc2_w).T))

    # rel_pos_bias as multiplicative exp(rpb), int4 unsigned w/ per-row scale
    NP2 = (N + 1) // 2
    rpbT = np.ascontiguousarray(
        np.asarray(inputs["rel_pos_bias"], f32)[0].transpose(0, 2, 1))
    Rp = np.zeros((RPB_PAD, 2 * NP2), f32)
    Rp[:RPB_ROWS, :N] = np.exp(rpbT.reshape(RPB_ROWS, N))
    rs = np.maximum(Rp.max(1), 1e-30) / 15.0
    Rq = np.rint(Rp / rs[:, None]).astype(np.uint8)
    rpb_pk = Rq[:, 0::2] | (Rq[:, 1::2] << 4)

    ws = np.concatenate([s_qk, s_v, s_pm, s_pc, s_f1, s_f2])   # 6912
    smA = np.concatenate([
        np.ascontiguousarray(qkb.reshape(12, P).T),
        np.ascontiguousarray(qkb.reshape(12, P).T),
        np.ascontiguousarray(fc1_b_eff.reshape(FT, P).T),
        np.ascontiguousarray(ws.reshape(54, P).T),
    ], axis=1).astype(f32)
    smB = np.stack([vb, gamma1 * proj_b, gamma1 * cov_proj_b,
                    gamma2 * fc2_b]).astype(f32)

    shared = {"smA": smA, "smB": smB}
    full = {
        "wqk_sh": p_qk,
        "wv_sh": p_v,
        "rpb_sh": rpb_pk,
        "rpbs_sh": rs.astype(f32).reshape(RPB_PAD, 1),
        "wpm_sh": p_pm,
        "wpc_sh": p_pc,
        "wfc1_sh": p_f1,
        "wfc2_sh": p_f2,
    }
    shards = {}
    for k, v in full.items():
        ch = v.shape[0] // NC
        shards[k] = [np.ascontiguousarray(v[c * ch:(c + 1) * ch]) for c in range(NC)]
    return shared, shards


def _quant_rows4(x):
    """Per-token symmetric int4 packed 2/byte: (p [N,D/2] u8, s [N] f32)."""
    amax = np.abs(x).max(axis=1)
    s = np.maximum(amax, 1e-30) / 7.0
    q8 = (np.rint(x / s[:, None]) + 8.0).astype(np.uint8)   # [1,15]
    p = q8[:, 0::2] | (q8[:, 1::2] << 4)
    return p, s.astype(np.float32)


def _prepare(inputs):
    """Build per-core input maps (host-side prep, outside the hot path)."""
    shared, shards = _prep_shared(inputs)
    xm = np.asarray(inputs["x_mean"], np.float32)
    xc = np.asarray(inputs["x_cov"], np.float32)

    in_maps = []
    for b in range(B):
        m = dict(shared)
        for k, lst in shards.items():
            m[k] = lst[b]
        qm, sm = _quant_rows4(xm[b])
        qc_, sc = _quant_rows4(xc[b])
        m["xm"] = qm
        m["xc"] = qc_
        m["xs"] = np.stack([sm, sc], 1)
        m["_xf32"] = (xm[b], xc[b])   # host-side residual base (not a param)
        in_maps.append(m)
    return in_maps


def _make_runner(nc, n_cores):
    """Cached-jit SPMD dispatcher (mirrors bass2jax.run_bass_via_pjrt, but the
    jit is built once, and the donated zero output buffers are created
    on-device instead of being shipped from the host every call)."""
    bass2jax.install_neuronx_cc_hook()
    assert nc.dbg_addr is None

    partition_name = nc.partition_id_tensor.name if nc.partition_id_tensor else None
    in_names, out_names, out_avals = [], [], []
    for alloc in nc.m.functions[0].allocations:
        if not isinstance(alloc, mybir.MemoryLocationSet):
            continue
        name = alloc.memorylocations[0].name
        if alloc.kind == "ExternalInput":
            if name != partition_name:
                in_names.append(name)
        elif alloc.kind == "ExternalOutput":
            shape = tuple(alloc.tensor_shape)
            dtype = mybir.dt.np(alloc.dtype)
            out_names.append(name)
            out_avals.append(jax.core.ShapedArray(shape, dtype))
    n_params = len(in_names)
    n_outs = len(out_names)
    param_names = list(in_names)
    in_names = in_names + out_names
    if partition_name is not None:
        in_names.append(partition_name)

    devices = jax.devices()[:n_cores]
    assert len(devices) == n_cores
    mesh = Mesh(np.asarray(devices), ("core",))
    donate = tuple(range(n_params, n_params + n_outs))

    def _body(*args):
        operands = list(args)
        if partition_name is not None:
            operands.append(bass2jax.partition_id_tensor())
        outs = bass2jax._bass_exec_p.bind(
            *operands,
            out_avals=tuple(out_avals),
            in_names=tuple(in_names),
            out_names=tuple(out_names),
            lowering_input_output_aliases=(),
            sim_require_finite=True,
            sim_require_nnan=True,
            nc=nc,
        )
        return tuple(outs)

    sharded = jax.jit(
        shard_map(_body, mesh=mesh,
                  in_specs=(PartitionSpec("core"),) * (n_params + n_outs),
                  out_specs=(PartitionSpec("core"),) * n_outs,
                  check_rep=False),
        donate_argnums=donate,
        keep_unused=True,
    )

    zero_shardings = tuple(NamedSharding(mesh, PartitionSpec("core"))
                           for _ in range(n_outs))

    def _zeros():
        return tuple(jnp.zeros((n_cores * a.shape[0], *a.shape[1:]), a.dtype)
                     for a in out_avals)

    zeros_jit = jax.jit(_zeros, out_shardings=zero_shardings)

    return {
        "sharded": sharded,
        "zeros_jit": zeros_jit,
        "param_names": param_names,
        "out_names": out_names,
        "out_avals": out_avals,
        "n_cores": n_cores,
    }


def _execute(in_maps):
    """One full dispatch: H2D, 8-core SPMD NEFF execution (with on-device
    AllGather of the sharded tensors), D2H, output assembly.

    The donated output buffers are recycled from the previous call (the
    kernel writes every output element, so their contents are irrelevant);
    only the first call pays for an on-device zeros launch. D2H pulls the
    8 per-device shards concurrently."""
    r = _CACHE["runner"]
    n_cores = r["n_cores"]
    per_core = [[np.asarray(m[name]) for name in r["param_names"]] for m in in_maps]
    concat_in = [
        np.concatenate([per_core[c][i] for c in range(n_cores)], axis=0)
        for i in range(len(r["param_names"]))
    ]
    don = _CACHE.pop("donate_bufs", None)
    if don is None:
        don = r["zeros_jit"]()
    out_arrs = r["sharded"](*concat_in, *don)
    _CACHE["donate_bufs"] = out_arrs

    pool = _CACHE.setdefault("d2h_pool", ThreadPoolExecutor(3 * n_cores))
    shards = []
    for o in out_arrs:
        byidx = sorted(o.addressable_shards, key=lambda s: s.index[0].start or 0)
        shards.extend(byidx)
    parts = list(pool.map(lambda s: np.asarray(s.data), shards))
    res = {}
    for i, name in enumerate(r["out_names"]):
        res[name] = np.stack(parts[i * n_cores:(i + 1) * n_cores], 0)
    # reconstruct y = x_f32 + dequant(unpack_int4(delta)), threaded per core
    ysc = res["ysc"].astype(np.float32)            # [B, N, 2]
    ym = np.empty((B, N, D), np.float32)
    yc = np.empty((B, N, D), np.float32)

    def rebuild(task):
        b, dst, xf, col = task
        p = res["y"][b][col * N:(col + 1) * N]
        q = np.empty((N, D), np.float32)
        q[:, 0::2] = (p & 15).astype(np.float32)
        q[:, 1::2] = (p >> 4).astype(np.float32)
        dst[b] = xf + (q - 8.0) * ysc[b, :, col:col + 1]

    list(pool.map(rebuild, [(b, dst, in_maps[b]["_xf32"][col], col)
                            for b in range(B)
                            for dst, col in ((ym, 0), (yc, 1))]))
    return ym, yc


def kernel(**inputs):
    if "nc" not in _CACHE:
        _CACHE["nc"] = _build_program()
        _CACHE["runner"] = _make_runner(_CACHE["nc"], NC)
    return _execute(_prepare(inputs))


# revision 62
# speedup vs baseline: 1.0303x; 1.0303x over previous
"""Trainium2 Bass kernel for the Wasserstein-attention transformer block.

Strategy: data-parallel over batch B=8 across 8 NeuronCores (one batch
element per core). Per core, the whole block runs with activations kept
in a transposed [feature, token] layout so every GEMM contracts over
partitions without runtime transposes of large tensors; attention runs
in S_T = [key, query] layout so softmax denominators and context
accumulation are plain matmuls. Matmul operands are bf16 (PSUM
accumulation fp32); the Wasserstein affine terms use f32r.

Host<->device traffic (the axon tunnel is ~35-65 MB/s and dominates the
wall clock) is minimized:
 - all shared tensors (weights + exp(rel_pos_bias)) are sent SHARDED 1/8
   per core and AllGather-ed on-device over the NeuronLink fabric;
 - weights and exp(rpb) travel as int4 (two nibbles per byte) with
   per-row f32 scales (gamma factors folded into the scales);
 - x inputs travel as int4 with per-token scales; the device returns
   only delta = y - x as per-token int4, and the host reconstructs
   y = x_f32 + dequant(delta), so the residual-stream precision never
   depends on the wire precision;
 - dispatch goes through a cached jit (no per-call retrace), donated
   output buffers are recycled from the previous call, and D2H pulls
   all shards concurrently.
"""
import contextlib
from concurrent.futures import ThreadPoolExecutor

import numpy as np
import ml_dtypes

import jax
import jax.numpy as jnp
from jax.experimental.shard_map import shard_map
from jax.sharding import Mesh, NamedSharding, PartitionSpec

import concourse.bass as bass
import concourse.tile as tile
from concourse import bacc, bass2jax, mybir
from concourse.masks import make_identity

F32 = mybir.dt.float32
F32R = mybir.dt.float32r
BF16 = mybir.dt.bfloat16
I8 = mybir.dt.int8
U8 = mybir.dt.uint8
F8 = mybir.dt.float8e4
AF = mybir.ActivationFunctionType
ALU = mybir.AluOpType
AXL = mybir.AxisListType

B, N, D, H = 8, 577, 768, 12
HD = D // H
DFF = 4 * D
SCALE = HD ** -0.5
LN_EPS = 1e-5

P = 128
NT = [(0, 128), (128, 128), (256, 128), (384, 128), (512, 65)]   # token tiles
QCH = [(0, 290), (290, 287)]                                     # psum-free chunks of N (both f32r-fast)
DT = D // P        # 6
FT = DFF // P      # 24
VCH = [(0, 384), (384, 384)]                                     # v / proj / fc2 out chunks

NC = 8             # cores
DSH = D // NC      # 96   row-shard of [D, *] weights
FSH = DFF // NC    # 384  row-shard of [DFF, *] weights
RPB_ROWS = H * N   # 6924
RPB_PAD = ((RPB_ROWS + NC - 1) // NC) * NC   # 6928
RPB_SH = RPB_PAD // NC                       # 866

GROUPS = [list(range(NC))]

_CACHE = {}


def _build_program():
    nc = bacc.Bacc("TRN2", target_bir_lowering=False, debug=False, num_devices=NC)

    # ---- DRAM I/O ----
    # per-core unique: int4 per-token-quantized x packed two-per-byte
    # (even col low nibble), scales in xs (col0=m, col1=c)
    xm_d = nc.declare_dram_parameter("xm", [N, D // 2], U8, isOutput=False)
    xc_d = nc.declare_dram_parameter("xc", [N, D // 2], U8, isOutput=False)
    xs_d = nc.declare_dram_parameter("xs", [N, 2], F32, isOutput=False)
    # sharded 1/8 per core, AllGather-ed on device; weights and exp(rpb)
    # travel as int4 packed two-per-byte with per-row f32 scales
    wqk_sh_d = nc.declare_dram_parameter("wqk_sh", [DSH, D], U8, isOutput=False)
    wv_sh_d = nc.declare_dram_parameter("wv_sh", [DSH, D // 2], U8, isOutput=False)
    rpb_sh_d = nc.declare_dram_parameter("rpb_sh", [RPB_SH, (N + 1) // 2], U8, isOutput=False)
    rpbs_sh_d = nc.declare_dram_parameter("rpbs_sh", [RPB_SH, 1], F32, isOutput=False)
    wpm_sh_d = nc.declare_dram_parameter("wpm_sh", [DSH, D // 2], U8, isOutput=False)
    wpc_sh_d = nc.declare_dram_parameter("wpc_sh", [DSH, D // 2], U8, isOutput=False)
    wfc1_sh_d = nc.declare_dram_parameter("wfc1_sh", [DSH, DFF // 2], U8, isOutput=False)
    wfc2_sh_d = nc.declare_dram_parameter("wfc2_sh", [FSH, D // 2], U8, isOutput=False)
    # small replicated f32: smA = qkbm(12) | qkbc(12) | fc1b(24) | wscales(54)
    # smB rows: vb, r1m, r1c, r2
    smA_d = nc.declare_dram_parameter("smA", [P, 102], F32, isOutput=False)
    smB_d = nc.declare_dram_parameter("smB", [4, D], F32, isOutput=False)
    # outputs: int4 per-token-quantized delta (y - x), two nibbles per byte
    # (even col in low nibble, odd in high); y rows 0..N-1 = mean stream,
    # N..2N-1 = cov stream; scales in ysc
    y_d = nc.declare_dram_parameter("y", [2 * N, D // 2], U8, isOutput=True)
    ysc_d = nc.declare_dram_parameter("ysc", [N, 2], F32, isOutput=True)

    with tile.TileContext(nc) as tc, contextlib.ExitStack() as top:
        # ---- on-device AllGather of the sharded shared tensors ----
        dram = top.enter_context(tc.tile_pool(name="dram_cc", bufs=1, space="DRAM"))

        def gathered(param, chunk, full, tag, dt=BF16):
            bt = dram.tile(list(chunk), dt, tag=f"b_{tag}", name=f"b_{tag}")
            gt = dram.tile(list(full), dt, tag=f"g_{tag}", name=f"g_{tag}")
            nc.sync.dma_start(out=bt[:], in_=param[:])
            nc.gpsimd.collective_compute(
                "AllGather", ALU.bypass, replica_groups=GROUPS,
                ins=[bt.opt()], outs=[gt.opt()])
            return gt

        # ordered by first use: qkv -> rpb -> proj -> mlp
        NP2 = (N + 1) // 2
        wqk_g = gathered(wqk_sh_d, (DSH, D), (D, D), "wqk", dt=U8)
        wv_g = gathered(wv_sh_d, (DSH, D // 2), (D, D // 2), "wv", dt=U8)
        rpb_g = gathered(rpb_sh_d, (RPB_SH, NP2), (RPB_PAD, NP2), "rpb", dt=U8)
        rpbs_g = gathered(rpbs_sh_d, (RPB_SH, 1), (RPB_PAD, 1), "rpbs", dt=F32)
        wpm_g = gathered(wpm_sh_d, (DSH, D // 2), (D, D // 2), "wpm", dt=U8)
        wpc_g = gathered(wpc_sh_d, (DSH, D // 2), (D, D // 2), "wpc", dt=U8)
        wfc1_g = gathered(wfc1_sh_d, (DSH, DFF // 2), (D, DFF // 2), "wfc1", dt=U8)
        wfc2_g = gathered(wfc2_sh_d, (FSH, D // 2), (DFF, D // 2), "wfc2", dt=U8)

        const = top.enter_context(tc.tile_pool(name="const", bufs=1))
        persist = top.enter_context(tc.tile_pool(name="persist", bufs=1))

        ident = const.tile([P, P], BF16, tag="ident", name="ident")
        make_identity(nc, ident)
        eps_t = const.tile([P, 1], F32, tag="eps", name="eps")
        nc.vector.memset(eps_t, LN_EPS)
        half_t = const.tile([P, 1], F32, tag="half", name="half")
        nc.vector.memset(half_t, 0.5)
        negh_f = const.tile([P, 2], F32, tag="negh_f", name="negh_f")
        nc.vector.memset(negh_f, -0.5)
        negh = const.tile([P, 2], F32R, tag="negh", name="negh")
        nc.vector.tensor_copy(out=negh[:], in_=negh_f[:])
        ones_f = const.tile([1, N], F32, tag="ones_f", name="ones_f")
        nc.vector.memset(ones_f, 1.0)
        ones_r = const.tile([1, N], F32R, tag="ones_r", name="ones_r")
        nc.vector.tensor_copy(out=ones_r[:], in_=ones_f[:])

        # biases / rows / weight scales, packed in smA / smB
        smA = persist.tile([P, 102], F32, tag="smA", name="smA")
        nc.sync.dma_start(out=smA[:], in_=smA_d[:])
        # smA columns: qkbm 0-11 | qkbc 12-23 | fc1b 24-47 | wscales 48-101
        # (wscales: wqk +0..5 | wv +6..11 | wpm +12..17 | wpc +18..23 |
        #  wfc1 +24..29 | wfc2 +30..53)
        vb_b = persist.tile([P, D], F32, tag="vb_b", name="vb_b")
        nc.sync.dma_start(out=vb_b[:], in_=smB_d[0:1, :].to_broadcast([P, D]))
        r1m_b = persist.tile([P, D], F32, tag="r1m_b", name="r1m_b")
        nc.sync.dma_start(out=r1m_b[:], in_=smB_d[1:2, :].to_broadcast([P, D]))
        r1c_b = persist.tile([P, D], F32, tag="r1c_b", name="r1c_b")
        nc.sync.dma_start(out=r1c_b[:], in_=smB_d[2:3, :].to_broadcast([P, D]))
        r2_b = persist.tile([P, D], F32, tag="r2_b", name="r2_b")
        nc.sync.dma_start(out=r2_b[:], in_=smB_d[3:4, :].to_broadcast([P, D]))

        # int4 -> bf16 weight loads: unpack nibbles, scale per in-feature row
        def load_w4(pool, pw, dst, src_ap, scale_ap, wp):
            st = pool.tile([P, pw], U8, tag="wst", name="wst")
            lo = pool.tile([P, pw], U8, tag="wlo", name="wlo")
            hi = pool.tile([P, pw], U8, tag="whi", name="whi")
            nc.sync.dma_start(out=st[:, :wp], in_=src_ap)
            nc.vector.tensor_scalar(out=lo[:, :wp], in0=st[:, :wp], scalar1=15,
                                    scalar2=None, op0=ALU.bitwise_and)
            nc.vector.tensor_scalar(out=hi[:, :wp], in0=st[:, :wp], scalar1=4,
                                    scalar2=None, op0=ALU.logical_shift_right)
            dg = dst.rearrange("p (d t) -> p t d", t=2)
            nc.vector.tensor_scalar(out=dg[:, 0, :], in0=lo[:, :wp], scalar1=8.0,
                                    scalar2=scale_ap, op0=ALU.subtract, op1=ALU.mult)
            nc.vector.tensor_scalar(out=dg[:, 1, :], in0=hi[:, :wp], scalar1=8.0,
                                    scalar2=scale_ap, op0=ALU.subtract, op1=ALU.mult)

        # residual-stream tiles (bf16, natural layout); become x' in place.
        # Loaded as int8 + per-token scale; delta tiles d_t accumulate the
        # gamma-scaled branch sums (the device output is delta = y - x).
        xs_t = []
        for i, (n0, nn) in enumerate(NT):
            t = persist.tile([P, 2], F32, tag=f"xs{i}", name=f"xs{i}")
            nc.sync.dma_start(out=t[:nn, :], in_=xs_d[n0:n0 + nn, :])
            xs_t.append(t)
        x_t, d_t = {}, {}
        xqp = top.enter_context(tc.tile_pool(name="xq", bufs=3))
        for si, (s, src) in enumerate((("m", xm_d), ("c", xc_d))):
            for i, (n0, nn) in enumerate(NT):
                xq = xqp.tile([P, D // 2], U8, tag="xq", name="xq")
                nc.sync.dma_start(out=xq[:nn, :], in_=src[n0:n0 + nn, :])
                lo = xqp.tile([P, D // 2], U8, tag="xlo", name="xlo")
                hi = xqp.tile([P, D // 2], U8, tag="xhi", name="xhi")
                nc.vector.tensor_scalar(out=lo[:nn, :], in0=xq[:nn, :], scalar1=15,
                                        scalar2=None, op0=ALU.bitwise_and)
                nc.vector.tensor_scalar(out=hi[:nn, :], in0=xq[:nn, :], scalar1=4,
                                        scalar2=None, op0=ALU.logical_shift_right)
                t = persist.tile([P, D], BF16, tag=f"x_{s}{i}", name=f"x_{s}{i}")
                tg = t[:nn, :].rearrange("p (d t) -> p t d", t=2)
                sc_ap = xs_t[i][:nn, si:si + 1]
                nc.vector.tensor_scalar(out=tg[:, 0, :], in0=lo[:nn, :], scalar1=8.0,
                                        scalar2=sc_ap, op0=ALU.subtract, op1=ALU.mult)
                nc.vector.tensor_scalar(out=tg[:, 1, :], in0=hi[:nn, :], scalar1=8.0,
                                        scalar2=sc_ap, op0=ALU.subtract, op1=ALU.mult)
                x_t[s, i] = t
                d_t[s, i] = persist.tile([P, D], BF16, tag=f"d_{s}{i}", name=f"d_{s}{i}")

        # ---------- helpers ----------
        def layernorm_transpose(lnp, psln, s, xhatT):
            """LN over feature dim of x_t[s,*] then transpose into xhatT[j] tiles."""
            for i, (n0, nn) in enumerate(NT):
                xt = x_t[s, i]
                stats = lnp.tile([P, 3, 6], F32, tag="stats", name="stats")
                xg = xt[:nn, :].rearrange("p (g d) -> p g d", g=3)
                for g in range(3):
                    nc.vector.bn_stats(out=stats[:nn, g, :], in_=xg[:, g, :])
                mv = lnp.tile([P, 2], F32, tag="mv", name="mv")
                nc.vector.bn_aggr(out=mv[:nn], in_=stats[:nn])
                rstd = lnp.tile([P, 1], F32, tag="rstd", name="rstd")
                nc.scalar.activation(out=rstd[:nn], in_=mv[:nn, 1:2], func=AF.Sqrt,
                                     bias=eps_t[:nn], scale=1.0)
                nc.vector.reciprocal(out=rstd[:nn], in_=rstd[:nn])
                xhat = lnp.tile([P, D], BF16, tag="xhat", name="xhat")
                nc.vector.tensor_scalar(out=xhat[:nn], in0=xt[:nn, :],
                                        scalar1=mv[:nn, 0:1], scalar2=rstd[:nn],
                                        op0=ALU.subtract, op1=ALU.mult)
                for j in range(DT):
                    pst = psln.tile([P, P], BF16, tag="pst", name="pst")
                    nc.tensor.transpose(out=pst[:, :nn], in_=xhat[:nn, j * P:(j + 1) * P],
                                        identity=ident[:nn, :nn])
                    if j % 2 == 0:
                        nc.scalar.copy(out=xhatT[j][:, n0:n0 + nn], in_=pst[:, :nn])
                    else:
                        nc.vector.tensor_copy(out=xhatT[j][:, n0:n0 + nn], in_=pst[:, :nn])

        # ================= Phase A/B: LN1 + QKV =================
        # Pool lifetimes are a stack (LIFO release): ctx_io spans A/B..D and is
        # opened first; attn_io spans A/B..C and closes right after attention.
        ctx_cm = tc.tile_pool(name="ctx_io", bufs=1)
        ctx_io = ctx_cm.__enter__()
        ctxm = [ctx_io.tile([P, N], BF16, tag=f"ctxm{j}", name=f"ctxm{j}") for j in range(DT)]
        ctxc = [ctx_io.tile([P, N], BF16, tag=f"ctxc{j}", name=f"ctxc{j}") for j in range(DT)]
        attn_cm = tc.tile_pool(name="attn_io", bufs=1)
        attn_io = attn_cm.__enter__()
        qc = [attn_io.tile([P, N], BF16, tag=f"qc{h}", name=f"qc{h}") for h in range(H)]
        kc = [attn_io.tile([P, N], BF16, tag=f"kc{h}", name=f"kc{h}") for h in range(H)]
        vm = [attn_io.tile([P, H, HD + 1], BF16, tag=f"vm{i}", name=f"vm{i}") for i in range(5)]
        vc = [attn_io.tile([P, H, HD], BF16, tag=f"vc{i}", name=f"vc{i}") for i in range(5)]
        for i, (n0, nn) in enumerate(NT):
            nc.vector.memset(vm[i][:nn, :, HD:HD + 1], 1.0)

        with contextlib.ExitStack() as ab:
            wpool = ab.enter_context(tc.tile_pool(name="wqkv", bufs=1))
            wqk = [wpool.tile([P, 2 * D], BF16, tag=f"wqk{j}", name=f"wqk{j}") for j in range(DT)]
            wv = [wpool.tile([P, D], BF16, tag=f"wv{j}", name=f"wv{j}") for j in range(DT)]
            wstg1 = ab.enter_context(tc.tile_pool(name="wstg1", bufs=2))
            for j in range(DT):
                load_w4(wstg1, D, wqk[j][:], wqk_g[j * P:(j + 1) * P, :],
                        smA[:, 48 + j:49 + j], D)
                load_w4(wstg1, D, wv[j][:], wv_g[j * P:(j + 1) * P, :],
                        smA[:, 54 + j:55 + j], D // 2)

            xhatT = {s: [wpool.tile([P, N], BF16, tag=f"xhatT_{s}{j}", name=f"xhatT_{s}{j}") for j in range(DT)]
                     for s in ("m", "c")}
            lnp1 = ab.enter_context(tc.tile_pool(name="ln_ln1", bufs=3))
            psln1 = ab.enter_context(tc.tile_pool(name="psln_ln1", bufs=2, space="PSUM"))
            for s in ("m", "c"):
                layernorm_transpose(lnp1, psln1, s, xhatT[s])

            psqk = ab.enter_context(tc.tile_pool(name="psqk", bufs=3, space="PSUM"))
            sc1 = ab.enter_context(tc.tile_pool(name="sc_covqk", bufs=3))

            # --- QK GEMMs, transposed layout out [d_out, n] ---
            for s in ("m", "c"):
                for t in range(2 * DT):           # 6 q-tiles then 6 k-tiles
                    is_q = t < DT
                    for (c0, cw) in QCH:
                        ps = psqk.tile([P, 512], F32, tag="ps", name="ps")
                        for j in range(DT):
                            nc.tensor.matmul(ps[:, :cw], lhsT=wqk[j][:, t * P:(t + 1) * P],
                                             rhs=xhatT[s][j][:, c0:c0 + cw],
                                             start=(j == 0), stop=(j == DT - 1))
                        hpair = (t % DT) * 2      # heads 2*(t%6), +1
                        dst = qc if is_q else kc
                        if s == "m":
                            # mean stream: out = sc*(z + b)
                            sc = SCALE if is_q else 1.0
                            for half in range(2):
                                pr = slice(64 * half, 64 * half + 64)
                                nc.vector.tensor_scalar(
                                    out=dst[hpair + half][0:64, c0:c0 + cw],
                                    in0=ps[pr, :cw], scalar1=smA[pr, t:t + 1],
                                    scalar2=sc, op0=ALU.add, op1=ALU.mult)
                        else:
                            # cov stream: c = sqrt(elu(z + b) + 1)
                            t1 = sc1.tile([P, 512], F32, tag="t1", name="t1")
                            nc.vector.tensor_scalar_add(out=t1[:, :cw], in0=ps[:, :cw],
                                                        scalar1=smA[:, 12 + t:13 + t])
                            t2 = sc1.tile([P, 512], F32, tag="t2", name="t2")
                            nc.vector.tensor_scalar_min(out=t2[:, :cw], in0=t1[:, :cw], scalar1=0.0)
                            nc.scalar.activation(out=t2[:, :cw], in_=t2[:, :cw], func=AF.Exp)
                            nc.vector.scalar_tensor_tensor(out=t1[:, :cw], in0=t1[:, :cw],
                                                           scalar=0.0, in1=t2[:, :cw],
                                                           op0=ALU.max, op1=ALU.add)
                            for half in range(2):
                                pr = slice(64 * half, 64 * half + 64)
                                nc.scalar.activation(
                                    out=dst[hpair + half][64:128, c0:c0 + cw],
                                    in_=t1[pr, :cw], func=AF.Sqrt)

            # --- V GEMMs, natural layout out [n, d_v] ---
            for s in ("m", "c"):
                for i, (n0, nn) in enumerate(NT):
                    for c2, (v0, vw) in enumerate(VCH):
                        ps = psqk.tile([P, 512], F32, tag="ps", name="ps")
                        for j in range(DT):
                            nc.tensor.matmul(ps[:nn, :vw], lhsT=xhatT[s][j][:, n0:n0 + nn],
                                             rhs=wv[j][:, v0:v0 + vw],
                                             start=(j == 0), stop=(j == DT - 1))
                        psg = ps[:nn, :vw].rearrange("p (g d) -> p g d", g=6)
                        vbg = vb_b[:nn, v0:v0 + vw].rearrange("p (g d) -> p g d", g=6)
                        hs = slice(6 * c2, 6 * c2 + 6)
                        if s == "m":
                            nc.vector.tensor_tensor(out=vm[i][:nn, hs, 0:HD], in0=psg,
                                                    in1=vbg, op=ALU.add)
                        else:
                            t1 = sc1.tile([P, 512], F32, tag="t1", name="t1")
                            t1g = t1[:nn, :vw].rearrange("p (g d) -> p g d", g=6)
                            nc.vector.tensor_tensor(out=t1g, in0=psg, in1=vbg, op=ALU.add)
                            t2 = sc1.tile([P, 512], F32, tag="t2", name="t2")
                            nc.vector.tensor_scalar_min(out=t2[:nn, :vw], in0=t1[:nn, :vw],
                                                        scalar1=0.0)
                            nc.scalar.activation(out=t2[:nn, :vw], in_=t2[:nn, :vw], func=AF.Exp)
                            t2g = t2[:nn, :vw].rearrange("p (g d) -> p g d", g=6)
                            nc.vector.scalar_tensor_tensor(out=vc[i][:nn, hs, :], in0=t1g,
                                                           scalar=0.0, in1=t2g,
                                                           op0=ALU.max, op1=ALU.add)

        # ================= Phase C: attention =================
        with contextlib.ExitStack() as at:
            AB = at.enter_context(tc.tile_pool(name="AB", bufs=1))
            # per-head K=2 affine operands packed at 32-aligned partition slots
            # (base partition must be 0/32/64): head h -> tile h//3,
            # partitions (h%3)*32 + {0,1}. A = [colterm; ones], B = [ones; rowterm]
            N2 = N + 1   # fp32r needs even innermost extents; pad column never read
            A_pack = [AB.tile([P, N2], F32R, tag=f"A_pack{t}", name=f"A_pack{t}") for t in range(4)]
            B_pack = [AB.tile([P, N2], F32R, tag=f"B_pack{t}", name=f"B_pack{t}") for t in range(4)]

            def ab_slot(h):
                return A_pack[h // 3], B_pack[h // 3], (h % 3) * 32
            sqp = at.enter_context(tc.tile_pool(name="sqp", bufs=2))
            stg = at.enter_context(tc.tile_pool(name="stg", bufs=2))
            sigp = at.enter_context(tc.tile_pool(name="sigp", bufs=5))
            rpbp = at.enter_context(tc.tile_pool(name="rpbp", bufs=5))
            ep = at.enter_context(tc.tile_pool(name="ep", bufs=12))
            denp = at.enter_context(tc.tile_pool(name="denp", bufs=2))
            rcb = at.enter_context(tc.tile_pool(name="rcb", bufs=2))
            ps_r = at.enter_context(tc.tile_pool(name="ps_r", bufs=2, space="PSUM"))
            ps_s = at.enter_context(tc.tile_pool(name="ps_s", bufs=2, space="PSUM"))
            ps_c = at.enter_context(tc.tile_pool(name="ps_c", bufs=1, space="PSUM"))

            for h in range(H):
                # affine terms: A=[ -0.5*|w_k|^2 ; 1 ], B=[ 1 ; -0.5*|u_q|^2 ]
                A_t, B_t, sl = ab_slot(h)
                nc.sync.dma_start(out=A_t[sl + 1:sl + 2, :N], in_=ones_r[:])
                nc.vector.tensor_copy(out=B_t[sl:sl + 1, :N], in_=ones_r[:])
                sq_k = sqp.tile([P, N2], F32R, tag="sq", name="sq")
                nc.vector.tensor_tensor(out=sq_k[:, :N], in0=kc[h][:], in1=kc[h][:], op=ALU.mult)
                for (c0, cw) in QCH:
                    cwe = cw + (cw % 2)
                    pr = ps_r.tile([2, 512], F32, tag="pr", name="pr")
                    nc.tensor.matmul(pr[:, :cwe], lhsT=negh[:], rhs=sq_k[:, c0:c0 + cwe],
                                     start=True, stop=True)
                    nc.scalar.copy(out=A_t[sl:sl + 1, c0:c0 + cw], in_=pr[0:1, :cw])
                sq_q = sqp.tile([P, N2], F32R, tag="sq", name="sq")
                nc.vector.tensor_tensor(out=sq_q[:, :N], in0=qc[h][:], in1=qc[h][:], op=ALU.mult)
                rowst = stg.tile([1, N], F32R, tag="rowst", name="rowst")
                for (c0, cw) in QCH:
                    cwe = cw + (cw % 2)
                    pr = ps_r.tile([2, 512], F32, tag="pr", name="pr")
                    nc.tensor.matmul(pr[:, :cwe], lhsT=negh[:], rhs=sq_q[:, c0:c0 + cwe],
                                     start=True, stop=True)
                    nc.scalar.copy(out=rowst[0:1, c0:c0 + cw], in_=pr[0:1, :cw])
                nc.sync.dma_start(out=B_t[sl + 1:sl + 2, :N], in_=rowst[:])

                # scores + sigmoid + rpb + exp, S_T layout [k, q]
                e_h, e2_h = [], []
                for kt, (k0, kn) in enumerate(NT):
                    r0 = h * N + k0
                    rpq = rpbp.tile([P, NP2], U8, tag="rpb", name="rpb")
                    nc.sync.dma_start(out=rpq[:kn, :], in_=rpb_g[r0:r0 + kn, :])
                    rps = rpbp.tile([P, 1], F32, tag="rps", name="rps")
                    nc.sync.dma_start(out=rps[:kn, :], in_=rpbs_g[r0:r0 + kn, :])
                    rlo = rpbp.tile([P, NP2], U8, tag="rlo", name="rlo")
                    rhi = rpbp.tile([P, NP2], U8, tag="rhi", name="rhi")
                    nc.vector.tensor_scalar(out=rlo[:kn, :], in0=rpq[:kn, :], scalar1=15,
                                            scalar2=None, op0=ALU.bitwise_and)
                    nc.vector.tensor_scalar(out=rhi[:kn, :], in0=rpq[:kn, :], scalar1=4,
                                            scalar2=None, op0=ALU.logical_shift_right)
                    rpb_b = rpbp.tile([P, 2 * NP2], BF16, tag="rpbb", name="rpbb")
                    rg = rpb_b[:kn, :].rearrange("p (d t) -> p t d", t=2)
                    nc.vector.tensor_scalar_mul(out=rg[:, 0, :], in0=rlo[:kn, :],
                                                scalar1=rps[:kn, 0:1])
                    nc.vector.tensor_scalar_mul(out=rg[:, 1, :], in0=rhi[:kn, :],
                                                scalar1=rps[:kn, 0:1])
                    sig = sigp.tile([P, N], F32, tag="sig", name="sig")
                    e_t = ep.tile([P, N], BF16, tag="e", name="e")
                    e2_t = ep.tile([P, N], BF16, tag="e2", name="e2")
                    for (c0, cw) in QCH:
                        ps = ps_s.tile([P, 512], F32, tag="ps", name="ps")
                        A_t, B_t, sl = ab_slot(h)
                        kne = kn + (kn % 2)
                        cwe = cw + (cw % 2)
                        nc.tensor.matmul(ps[:kn, :cw], lhsT=kc[h][:, k0:k0 + kn],
                                         rhs=qc[h][:, c0:c0 + cw], start=True, stop=False)
                        nc.tensor.matmul(ps[:kne, :cwe], lhsT=A_t[sl:sl + 2, k0:k0 + kne],
                                         rhs=B_t[sl:sl + 2, c0:c0 + cwe], start=False, stop=True,
                                         skip_group_check=True)
                        # sigmoid(2x) = 0.5*tanh(x) + 0.5; tanh shares the ACT
                        # table set with exp.
                        nc.scalar.activation(out=sig[:kn, c0:c0 + cw], in_=ps[:kn, :cw],
                                             func=AF.Tanh, scale=1.0)
                    # e = exp(0.5*tanh + 0.5) * exp(rpb)   (rpb sent as fp8 factor)
                    nc.scalar.activation(out=sig[:kn, :], in_=sig[:kn, :], func=AF.Exp,
                                         bias=half_t[:kn], scale=0.5)
                    nc.vector.tensor_tensor(out=e_t[:kn, :], in0=sig[:kn, :],
                                            in1=rpb_b[:kn, :N], op=ALU.mult)
                    nc.gpsimd.tensor_tensor(out=e2_t[:kn, :], in0=e_t[:kn, :],
                                            in1=e_t[:kn, :], op=ALU.mult)
                    e_h.append(e_t)
                    e2_h.append(e2_t)

                # context matmuls (unnormalized) + per-chunk denominator:
                # each chunk's reciprocal/broadcast/evict chain depends only on
                # its own denominator slice, so chunks (and heads) pipeline.
                den = denp.tile([1, N], F32, tag="den", name="den")
                recip = denp.tile([1, N], F32, tag="recip", name="recip")
                rb = rcb.tile([64, N], F32, tag="rb", name="rb")
                rb2 = rcb.tile([64, N], F32, tag="rb2", name="rb2")
                jt, rr = h // 2, slice(64 * (h % 2), 64 * (h % 2) + 64)
                for ci, (c0, cw) in enumerate(QCH):
                    pm = ps_c.tile([65, 512], F32, tag=f"pcm{ci}", name=f"pcm{ci}")
                    pc2 = ps_c.tile([64, 512], F32, tag=f"pcc{ci}", name=f"pcc{ci}")
                    for kt, (k0, kn) in enumerate(NT):
                        nc.tensor.matmul(pm[:, :cw], lhsT=vm[kt][:kn, h, :],
                                         rhs=e_h[kt][:kn, c0:c0 + cw],
                                         start=(kt == 0), stop=(kt == 4))
                        nc.tensor.matmul(pc2[:, :cw], lhsT=vc[kt][:kn, h, :],
                                         rhs=e2_h[kt][:kn, c0:c0 + cw],
                                         start=(kt == 0), stop=(kt == 4))
                    nc.scalar.copy(out=den[0:1, c0:c0 + cw], in_=pm[64:65, :cw])
                    nc.vector.reciprocal(out=recip[0:1, c0:c0 + cw],
                                         in_=den[0:1, c0:c0 + cw])
                    nc.gpsimd.partition_broadcast(rb[:, c0:c0 + cw],
                                                  recip[0:1, c0:c0 + cw])
                    nc.vector.tensor_tensor(out=rb2[:, c0:c0 + cw],
                                            in0=rb[:, c0:c0 + cw],
                                            in1=rb[:, c0:c0 + cw], op=ALU.mult)
                    nc.vector.tensor_tensor(out=ctxm[jt][rr, c0:c0 + cw],
                                            in0=pm[0:64, :cw],
                                            in1=rb[:, c0:c0 + cw], op=ALU.mult)
                    nc.vector.tensor_tensor(out=ctxc[jt][rr, c0:c0 + cw],
                                            in0=pc2[0:64, :cw],
                                            in1=rb2[:, c0:c0 + cw], op=ALU.mult)

        attn_cm.__exit__(None, None, None)

        # ================= Phase D: proj + residual =================
        with contextlib.ExitStack() as pd:
            wpp = pd.enter_context(tc.tile_pool(name="wproj", bufs=1))
            wpm = [wpp.tile([P, D], BF16, tag=f"wpm{j}", name=f"wpm{j}") for j in range(DT)]
            wpc = [wpp.tile([P, D], BF16, tag=f"wpc{j}", name=f"wpc{j}") for j in range(DT)]
            wstg2 = pd.enter_context(tc.tile_pool(name="wstg2", bufs=2))
            for j in range(DT):
                load_w4(wstg2, D // 2, wpm[j][:], wpm_g[j * P:(j + 1) * P, :],
                        smA[:, 60 + j:61 + j], D // 2)
                load_w4(wstg2, D // 2, wpc[j][:], wpc_g[j * P:(j + 1) * P, :],
                        smA[:, 66 + j:67 + j], D // 2)
            psp = pd.enter_context(tc.tile_pool(name="psproj", bufs=3, space="PSUM"))
            for s, ctx_t, wp, rb_row in (("m", ctxm, wpm, r1m_b), ("c", ctxc, wpc, r1c_b)):
                for i, (n0, nn) in enumerate(NT):
                    for (v0, vw) in VCH:
                        ps = psp.tile([P, 512], F32, tag="ps", name="ps")
                        for j in range(DT):
                            nc.tensor.matmul(ps[:nn, :vw], lhsT=ctx_t[j][:, n0:n0 + nn],
                                             rhs=wp[j][:, v0:v0 + vw],
                                             start=(j == 0), stop=(j == DT - 1))
                        xt, dt = x_t[s, i], d_t[s, i]
                        # dd = gamma1*proj(ctx) + r1  (gamma folded into w-scales)
                        nc.vector.tensor_tensor(out=dt[:nn, v0:v0 + vw],
                                                in0=ps[:nn, :vw],
                                                in1=rb_row[:nn, v0:v0 + vw], op=ALU.add)
                        nc.vector.tensor_tensor(out=xt[:nn, v0:v0 + vw],
                                                in0=xt[:nn, v0:v0 + vw],
                                                in1=dt[:nn, v0:v0 + vw], op=ALU.add)

        ctx_cm.__exit__(None, None, None)

        # ================= Phase E/F: LN2 + MLP =================
        with contextlib.ExitStack() as pf:
            wfp = pf.enter_context(tc.tile_pool(name="wfc", bufs=1))
            wfc1 = [wfp.tile([P, DFF], BF16, tag=f"wfc1_{j}", name=f"wfc1_{j}") for j in range(DT)]
            wstg3 = pf.enter_context(tc.tile_pool(name="wstg3", bufs=2))
            for j in range(DT):
                load_w4(wstg3, DFF // 2, wfc1[j][:], wfc1_g[j * P:(j + 1) * P, :],
                        smA[:, 72 + j:73 + j], DFF // 2)
            wfc2 = [wfp.tile([P, D], BF16, tag=f"wfc2_{f}", name=f"wfc2_{f}") for f in range(FT)]
            for f in range(FT):
                load_w4(wstg3, DFF // 2, wfc2[f][:], wfc2_g[f * P:(f + 1) * P, :],
                        smA[:, 78 + f:79 + f], D // 2)

            xhat2T = {s: [wfp.tile([P, N], BF16, tag=f"xh2T_{s}{j}", name=f"xh2T_{s}{j}") for j in range(DT)]
                      for s in ("m", "c")}
            lnp2 = pf.enter_context(tc.tile_pool(name="ln_ln2", bufs=3))
            psln2 = pf.enter_context(tc.tile_pool(name="psln_ln2", bufs=2, space="PSUM"))
            for s in ("m", "c"):
                layernorm_transpose(lnp2, psln2, s, xhat2T[s])

            psf = pf.enter_context(tc.tile_pool(name="psfc", bufs=4, space="PSUM"))
            hp = pf.enter_context(tc.tile_pool(name="hT", bufs=1))
            outp = pf.enter_context(tc.tile_pool(name="outp", bufs=3))
            for si, s in enumerate(("m", "c")):
                # hT tiles shared between streams (tag reuse serializes via deps)
                hT = {s: [hp.tile([P, N], BF16, tag=f"hT{f}", name=f"hT{f}")
                          for f in range(FT)]}
                for f in range(FT):
                    for (c0, cw) in QCH:
                        ps = psf.tile([P, 512], F32, tag="ps", name="ps")
                        for j in range(DT):
                            nc.tensor.matmul(ps[:, :cw], lhsT=wfc1[j][:, f * P:(f + 1) * P],
                                             rhs=xhat2T[s][j][:, c0:c0 + cw],
                                             start=(j == 0), stop=(j == DT - 1))
                        nc.scalar.activation(out=hT[s][f][:, c0:c0 + cw], in_=ps[:, :cw],
                                             func=AF.Gelu, bias=smA[:, 24 + f:25 + f],
                                             scale=1.0)
                for i, (n0, nn) in enumerate(NT):
                    # delta = d_attn + gamma2*mlp(...) + r2; quantize per token
                    yt = outp.tile([P, D], F32, tag="yt", name="yt")
                    yq = outp.tile([P, D], U8, tag="yq", name="yq")
                    for (v0, vw) in VCH:
                        ps = psf.tile([P, 512], F32, tag="ps", name="ps")
                        for f in range(FT):
                            nc.tensor.matmul(ps[:nn, :vw], lhsT=hT[s][f][:, n0:n0 + nn],
                                             rhs=wfc2[f][:, v0:v0 + vw],
                                             start=(f == 0), stop=(f == FT - 1))
                        # gamma2*fc2(h) + d_attn  (gamma folded into w-scales)
                        nc.vector.tensor_tensor(out=yt[:nn, v0:v0 + vw], in0=ps[:nn, :vw],
                                                in1=d_t[s, i][:nn, v0:v0 + vw], op=ALU.add)
                        nc.vector.tensor_tensor(out=yt[:nn, v0:v0 + vw],
                                                in0=yt[:nn, v0:v0 + vw],
                                                in1=r2_b[:nn, v0:v0 + vw], op=ALU.add)
                    am = outp.tile([P, 1], F32, tag="am", name="am")
                    nc.vector.tensor_reduce(out=am[:nn], in_=yt[:nn, :], axis=AXL.X,
                                            op=ALU.max, apply_absolute_value=True)
                    sc_t = outp.tile([P, 1], F32, tag="sc", name="sc")
                    nc.vector.tensor_scalar(out=sc_t[:nn], in0=am[:nn],
                                            scalar1=1.0 / 7.0, scalar2=1e-30,
                                            op0=ALU.mult, op1=ALU.max)
                    inv = outp.tile([P, 1], F32, tag="inv", name="inv")
                    nc.vector.reciprocal(out=inv[:nn], in_=sc_t[:nn])
                    # biased nibble q+8 in [1,15], then pack pairs: lo|hi<<4
                    nc.vector.tensor_scalar(out=yq[:nn, :], in0=yt[:nn, :],
                                            scalar1=inv[:nn], scalar2=8.0,
                                            op0=ALU.mult, op1=ALU.add)
                    qg = yq[:nn, :].rearrange("p (d t) -> p t d", t=2)
                    pk = outp.tile([P, D // 2], U8, tag="pk", name="pk")
                    nc.vector.scalar_tensor_tensor(out=pk[:nn, :], in0=qg[:, 1, :],
                                                   scalar=16.0, in1=qg[:, 0, :],
                                                   op0=ALU.mult, op1=ALU.add)
                    nc.sync.dma_start(out=y_d[si * N + n0:si * N + n0 + nn, :],
                                      in_=pk[:nn, :])
                    nc.sync.dma_start(out=ysc_d[n0:n0 + nn, si:si + 1], in_=sc_t[:nn])

    nc.compile()
    return nc


def _prep_shared(inputs):
    f32 = np.float32
    g = lambda k: np.asarray(inputs[k], f32)
    qkv_w, norm1_w, norm1_b = g("qkv_w"), g("norm1_w"), g("norm1_b")
    qkv_w_eff = qkv_w * norm1_w[None, :]
    qkv_b_eff = qkv_w_eff @ norm1_b

    wqkT = np.ascontiguousarray(qkv_w_eff[:2 * D].T)
    wvT = np.ascontiguousarray(qkv_w_eff[2 * D:].T)
    qkb = qkv_b_eff[:2 * D]
    vb = qkv_b_eff[2 * D:]

    gamma1, gamma2 = g("gamma1"), g("gamma2")
    proj_w, proj_b = g("proj_w"), g("proj_b")
    cov_proj_w, cov_proj_b = g("cov_proj_w"), g("cov_proj_b")
    norm2_w, norm2_b = g("norm2_w"), g("norm2_b")
    fc1_w, fc1_b = g("fc1_w"), g("fc1_b")
    fc2_w, fc2_b = g("fc2_w"), g("fc2_b")

    fc1_w_eff = fc1_w * norm2_w[None, :]
    fc1_b_eff = fc1_b + fc1_w_eff @ norm2_b

    def pack_w(Wt):
        """Per-row symmetric int4, nibble-packed; returns (u8 [r, c/2], s [r])."""
        amax = np.abs(Wt).max(1)
        s = np.maximum(amax, 1e-30) / 7.0
        q8 = (np.rint(Wt / s[:, None]) + 8.0).astype(np.uint8)
        return q8[:, 0::2] | (q8[:, 1::2] << 4), s.astype(f32)

    p_qk, s_qk = pack_w(wqkT)
    p_v, s_v = pack_w(wvT)
    p_pm, s_pm = pack_w(np.ascontiguousarray((gamma1[:, None] * proj_w).T))
    p_pc, s_pc = pack_w(np.ascontiguousarray((gamma1[:, None] * cov_proj_w).T))
    p_f1, s_f1 = pack_w(np.ascontiguousarray(fc1_w_eff.T))
    p_f2, s_f2 = pack_w(np.ascontiguousarray((gamma2[:, None] * fc2_w).T))

    # rel_pos_bias as multiplicative exp(rpb), int4 unsigned w/ per-row scale
    NP2 = (N + 1) // 2
    rpbT = np.ascontiguousarray(
        np.asarray(inputs["rel_pos_bias"], f32)[0].transpose(0, 2, 1))
    Rp = np.zeros((RPB_PAD, 2 * NP2), f32)
    Rp[:RPB_ROWS, :N] = np.exp(rpbT.reshape(RPB_ROWS, N))
    rs = np.maximum(Rp.max(1), 1e-30) / 15.0
    Rq = np.rint(Rp / rs[:, None]).astype(np.uint8)
    rpb_pk = Rq[:, 0::2] | (Rq[:, 1::2] << 4)

    ws = np.concatenate([s_qk, s_v, s_pm, s_pc, s_f1, s_f2])   # 6912
    smA = np.concatenate([
        np.ascontiguousarray(qkb.reshape(12, P).T),
        np.ascontiguousarray(qkb.reshape(12, P).T),
        np.ascontiguousarray(fc1_b_eff.reshape(FT, P).T),
        np.ascontiguousarray(ws.reshape(54, P).T),
    ], axis=1).astype(f32)
    smB = np.stack([vb, gamma1 * proj_b, gamma1 * cov_proj_b,
                    gamma2 * fc2_b]).astype(f32)

    shared = {"smA": smA, "smB": smB}
    full = {
        "wqk_sh": p_qk,
        "wv_sh": p_v,
        "rpb_sh": rpb_pk,
        "rpbs_sh": rs.astype(f32).reshape(RPB_PAD, 1),
        "wpm_sh": p_pm,
        "wpc_sh": p_pc,
        "wfc1_sh": p_f1,
        "wfc2_sh": p_f2,
    }
    shards = {}
    for k, v in full.items():
        ch = v.shape[0] // NC
        shards[k] = [np.ascontiguousarray(v[c * ch:(c + 1) * ch]) for c in range(NC)]
    return shared, shards


def _quant_rows4(x):
    """Per-token symmetric int4 packed 2/byte: (p [N,D/2] u8, s [N] f32)."""
    amax = np.abs(x).max(axis=1)
    s = np.maximum(amax, 1e-30) / 7.0
    q8 = (np.rint(x / s[:, None]) + 8.0).astype(np.uint8)   # [1,15]
    p = q8[:, 0::2] | (q8[:, 1::2] << 4)
    return p, s.astype(np.float32)


def _prepare(inputs):
    """Build per-core input maps (host-side prep, outside the hot path)."""
    shared, shards = _prep_shared(inputs)
    xm = np.asarray(inputs["x_mean"], np.float32)
    xc = np.asarray(inputs["x_cov"], np.float32)

    in_maps = []
    for b in range(B):
        m = dict(shared)
        for k, lst in shards.items():
            m[k] = lst[b]
        qm, sm = _quant_rows4(xm[b])
        qc_, sc = _quant_rows4(xc[b])
        m["xm"] = qm
        m["xc"] = qc_
        m["xs"] = np.stack([sm, sc], 1)
        m["_xf32"] = (xm[b], xc[b])   # host-side residual base (not a param)
        in_maps.append(m)
    return in_maps


def _make_runner(nc, n_cores):
    """Cached-jit SPMD dispatcher (mirrors bass2jax.run_bass_via_pjrt, but the
    jit is built once, and the donated zero output buffers are created
    on-device instead of being shipped from the host every call)."""
    bass2jax.install_neuronx_cc_hook()
    assert nc.dbg_addr is None

    partition_name = nc.partition_id_tensor.name if nc.partition_id_tensor else None
    in_names, out_names, out_avals, in_shapes = [], [], [], []
    for alloc in nc.m.functions[0].allocations:
        if not isinstance(alloc, mybir.MemoryLocationSet):
            continue
        name = alloc.memorylocations[0].name
        if alloc.kind == "ExternalInput":
            if name != partition_name:
                in_names.append(name)
                in_shapes.append((tuple(alloc.tensor_shape), mybir.dt.np(alloc.dtype)))
        elif alloc.kind == "ExternalOutput":
            shape = tuple(alloc.tensor_shape)
            dtype = mybir.dt.np(alloc.dtype)
            out_names.append(name)
            out_avals.append(jax.core.ShapedArray(shape, dtype))
    n_params = len(in_names)
    n_outs = len(out_names)
    param_names = list(in_names)
    in_names = in_names + out_names
    if partition_name is not None:
        in_names.append(partition_name)

    devices = jax.devices()[:n_cores]
    assert len(devices) == n_cores
    mesh = Mesh(np.asarray(devices), ("core",))
    donate = tuple(range(n_params, n_params + n_outs))

    def _body(*args):
        operands = list(args)
        if partition_name is not None:
            operands.append(bass2jax.partition_id_tensor())
        outs = bass2jax._bass_exec_p.bind(
            *operands,
            out_avals=tuple(out_avals),
            in_names=tuple(in_names),
            out_names=tuple(out_names),
            lowering_input_output_aliases=(),
            sim_require_finite=True,
            sim_require_nnan=True,
            nc=nc,
        )
        return tuple(outs)

    sm = shard_map(_body, mesh=mesh,
                   in_specs=(PartitionSpec("core"),) * (n_params + n_outs),
                   out_specs=(PartitionSpec("core"),) * n_outs,
                   check_rep=False)
    try:
        # AOT compile with bass_effect suppressed -> C++ fast-path dispatch
        example = [jax.ShapeDtypeStruct((n_cores * s[0], *s[1:]), dt)
                   for s, dt in in_shapes]
        example += [jax.ShapeDtypeStruct((n_cores * a.shape[0], *a.shape[1:]), a.dtype)
                    for a in out_avals]
        sharded = bass2jax.fast_dispatch_compile(
            lambda: jax.jit(sm, donate_argnums=donate, keep_unused=True)
            .lower(*example).compile())
    except Exception:
        sharded = jax.jit(sm, donate_argnums=donate, keep_unused=True)

    zero_shardings = tuple(NamedSharding(mesh, PartitionSpec("core"))
                           for _ in range(n_outs))

    def _zeros():
        return tuple(jnp.zeros((n_cores * a.shape[0], *a.shape[1:]), a.dtype)
                     for a in out_avals)

    zeros_jit = jax.jit(_zeros, out_shardings=zero_shardings)

    return {
        "sharded": sharded,
        "zeros_jit": zeros_jit,
        "param_names": param_names,
        "out_names": out_names,
        "out_avals": out_avals,
        "n_cores": n_cores,
    }


def _execute(in_maps):
    """One full dispatch: H2D, 8-core SPMD NEFF execution (with on-device
    AllGather of the sharded tensors), D2H, output assembly.

    The donated output buffers are recycled from the previous call (the
    kernel writes every output element, so their contents are irrelevant);
    only the first call pays for an on-device zeros launch. D2H pulls the
    8 per-device shards concurrently."""
    r = _CACHE["runner"]
    n_cores = r["n_cores"]
    per_core = [[np.asarray(m[name]) for name in r["param_names"]] for m in in_maps]
    concat_in = [
        np.concatenate([per_core[c][i] for c in range(n_cores)], axis=0)
        for i in range(len(r["param_names"]))
    ]
    don = _CACHE.pop("donate_bufs", None)
    if don is None:
        don = r["zeros_jit"]()
    out_arrs = r["sharded"](*concat_in, *don)
    _CACHE["donate_bufs"] = out_arrs

    pool = _CACHE.setdefault("d2h_pool", ThreadPoolExecutor(3 * n_cores))
    shards = []
    for o in out_arrs:
        byidx = sorted(o.addressable_shards, key=lambda s: s.index[0].start or 0)
        shards.extend(byidx)
    parts = list(pool.map(lambda s: np.asarray(s.data), shards))
    res = {}
    for i, name in enumerate(r["out_names"]):
        res[name] = np.stack(parts[i * n_cores:(i + 1) * n_cores], 0)
    # reconstruct y = x_f32 + dequant(unpack_int4(delta)), threaded per core
    ysc = res["ysc"].astype(np.float32)            # [B, N, 2]
    ym = np.empty((B, N, D), np.float32)
    yc = np.empty((B, N, D), np.float32)

    def rebuild(task):
        b, dst, xf, col = task
        p = res["y"][b][col * N:(col + 1) * N]
        q = np.empty((N, D), np.float32)
        q[:, 0::2] = (p & 15).astype(np.float32)
        q[:, 1::2] = (p >> 4).astype(np.float32)
        dst[b] = xf + (q - 8.0) * ysc[b, :, col:col + 1]

    list(pool.map(rebuild, [(b, dst, in_maps[b]["_xf32"][col], col)
                            for b in range(B)
                            for dst, col in ((ym, 0), (yc, 1))]))
    return ym, yc


def kernel(**inputs):
    if "nc" not in _CACHE:
        _CACHE["nc"] = _build_program()
        _CACHE["runner"] = _make_runner(_CACHE["nc"], NC)
    return _execute(_prepare(inputs))


# revision 63
# speedup vs baseline: 1.0876x; 1.0556x over previous
"""Trainium2 Bass kernel for the Wasserstein-attention transformer block.

Strategy: data-parallel over batch B=8 across 8 NeuronCores (one batch
element per core). Per core, the whole block runs with activations kept
in a transposed [feature, token] layout so every GEMM contracts over
partitions without runtime transposes of large tensors; attention runs
in S_T = [key, query] layout so softmax denominators and context
accumulation are plain matmuls. Matmul operands are bf16 (PSUM
accumulation fp32); the Wasserstein affine terms use f32r.

Host<->device traffic (the axon tunnel is ~35-65 MB/s and dominates the
wall clock) is minimized:
 - all shared tensors (weights + exp(rel_pos_bias)) are sent SHARDED 1/8
   per core and AllGather-ed on-device over the NeuronLink fabric;
 - weights and exp(rpb) travel as int4 (two nibbles per byte) with
   per-row f32 scales (gamma factors folded into the scales);
 - x inputs travel as int4 with per-token scales; the device returns
   only delta = y - x as per-token int4, and the host reconstructs
   y = x_f32 + dequant(delta), so the residual-stream precision never
   depends on the wire precision;
 - dispatch goes through a cached jit (no per-call retrace), donated
   output buffers are recycled from the previous call, and D2H pulls
   all shards concurrently.
"""
import contextlib
from concurrent.futures import ThreadPoolExecutor

import numpy as np
import ml_dtypes

import jax
import jax.numpy as jnp
from jax.experimental.shard_map import shard_map
from jax.sharding import Mesh, NamedSharding, PartitionSpec

import concourse.bass as bass
import concourse.tile as tile
from concourse import bacc, bass2jax, mybir
from concourse.masks import make_identity

F32 = mybir.dt.float32
F32R = mybir.dt.float32r
BF16 = mybir.dt.bfloat16
I8 = mybir.dt.int8
U8 = mybir.dt.uint8
F8 = mybir.dt.float8e4
AF = mybir.ActivationFunctionType
ALU = mybir.AluOpType
AXL = mybir.AxisListType

B, N, D, H = 8, 577, 768, 12
HD = D // H
DFF = 4 * D
SCALE = HD ** -0.5
LN_EPS = 1e-5

P = 128
NT = [(0, 128), (128, 128), (256, 128), (384, 128), (512, 65)]   # token tiles
QCH = [(0, 290), (290, 287)]                                     # psum-free chunks of N (both f32r-fast)
DT = D // P        # 6
FT = DFF // P      # 24
VCH = [(0, 384), (384, 384)]                                     # v / proj / fc2 out chunks

NC = 8             # cores
DSH = D // NC      # 96   row-shard of [D, *] weights
FSH = DFF // NC    # 384  row-shard of [DFF, *] weights
RPB_ROWS = H * N   # 6924
RPB_PAD = ((RPB_ROWS + NC - 1) // NC) * NC   # 6928
RPB_SH = RPB_PAD // NC                       # 866

GROUPS = [list(range(NC))]

_CACHE = {}


def _build_program():
    nc = bacc.Bacc("TRN2", target_bir_lowering=False, debug=False, num_devices=NC)

    # ---- DRAM I/O ----
    # per-core unique: int4 per-token-quantized x packed two-per-byte
    # (even col low nibble), scales in xs (col0=m, col1=c)
    xm_d = nc.declare_dram_parameter("xm", [N, D // 2], U8, isOutput=False)
    xc_d = nc.declare_dram_parameter("xc", [N, D // 2], U8, isOutput=False)
    xs_d = nc.declare_dram_parameter("xs", [N, 2], F32, isOutput=False)
    # sharded 1/8 per core, AllGather-ed on device; weights and exp(rpb)
    # travel as int4 packed two-per-byte with per-row f32 scales
    wqk_sh_d = nc.declare_dram_parameter("wqk_sh", [DSH, D], U8, isOutput=False)
    wv_sh_d = nc.declare_dram_parameter("wv_sh", [DSH, D // 2], U8, isOutput=False)
    rpb_sh_d = nc.declare_dram_parameter("rpb_sh", [RPB_SH, (N + 1) // 2], U8, isOutput=False)
    rpbs_sh_d = nc.declare_dram_parameter("rpbs_sh", [RPB_SH, 1], F32, isOutput=False)
    wpm_sh_d = nc.declare_dram_parameter("wpm_sh", [DSH, D // 2], U8, isOutput=False)
    wpc_sh_d = nc.declare_dram_parameter("wpc_sh", [DSH, D // 2], U8, isOutput=False)
    wfc1_sh_d = nc.declare_dram_parameter("wfc1_sh", [DSH, DFF // 2], U8, isOutput=False)
    wfc2_sh_d = nc.declare_dram_parameter("wfc2_sh", [FSH, D // 2], U8, isOutput=False)
    # smA = qkbm(12) | qkbc(12) | fc1b(24) | wscales(54), row-sharded 1/8
    # smB rows: vb, r1m, r1c, r2 (replicated, tiny)
    smA_sh_d = nc.declare_dram_parameter("smA_sh", [P // NC, 102], F32, isOutput=False)
    smB_d = nc.declare_dram_parameter("smB", [4, D], F32, isOutput=False)
    # outputs: int4 per-token-quantized delta (y - x), two nibbles per byte
    # (even col in low nibble, odd in high); y rows 0..N-1 = mean stream,
    # N..2N-1 = cov stream; scales in ysc
    y_d = nc.declare_dram_parameter("y", [2 * N, D // 2], U8, isOutput=True)
    ysc_d = nc.declare_dram_parameter("ysc", [N, 2], F32, isOutput=True)

    with tile.TileContext(nc) as tc, contextlib.ExitStack() as top:
        # ---- on-device AllGather of the sharded shared tensors ----
        dram = top.enter_context(tc.tile_pool(name="dram_cc", bufs=1, space="DRAM"))

        def gathered(param, chunk, full, tag, dt=BF16):
            bt = dram.tile(list(chunk), dt, tag=f"b_{tag}", name=f"b_{tag}")
            gt = dram.tile(list(full), dt, tag=f"g_{tag}", name=f"g_{tag}")
            nc.sync.dma_start(out=bt[:], in_=param[:])
            nc.gpsimd.collective_compute(
                "AllGather", ALU.bypass, replica_groups=GROUPS,
                ins=[bt.opt()], outs=[gt.opt()])
            return gt

        # ordered by first use: qkv -> rpb -> proj -> mlp
        NP2 = (N + 1) // 2
        wqk_g = gathered(wqk_sh_d, (DSH, D), (D, D), "wqk", dt=U8)
        wv_g = gathered(wv_sh_d, (DSH, D // 2), (D, D // 2), "wv", dt=U8)
        rpb_g = gathered(rpb_sh_d, (RPB_SH, NP2), (RPB_PAD, NP2), "rpb", dt=U8)
        rpbs_g = gathered(rpbs_sh_d, (RPB_SH, 1), (RPB_PAD, 1), "rpbs", dt=F32)
        wpm_g = gathered(wpm_sh_d, (DSH, D // 2), (D, D // 2), "wpm", dt=U8)
        wpc_g = gathered(wpc_sh_d, (DSH, D // 2), (D, D // 2), "wpc", dt=U8)
        wfc1_g = gathered(wfc1_sh_d, (DSH, DFF // 2), (D, DFF // 2), "wfc1", dt=U8)
        wfc2_g = gathered(wfc2_sh_d, (FSH, D // 2), (DFF, D // 2), "wfc2", dt=U8)
        smA_g = gathered(smA_sh_d, (P // NC, 102), (P, 102), "smA", dt=F32)

        const = top.enter_context(tc.tile_pool(name="const", bufs=1))
        persist = top.enter_context(tc.tile_pool(name="persist", bufs=1))

        ident = const.tile([P, P], BF16, tag="ident", name="ident")
        make_identity(nc, ident)
        eps_t = const.tile([P, 1], F32, tag="eps", name="eps")
        nc.vector.memset(eps_t, LN_EPS)
        half_t = const.tile([P, 1], F32, tag="half", name="half")
        nc.vector.memset(half_t, 0.5)
        negh_f = const.tile([P, 2], F32, tag="negh_f", name="negh_f")
        nc.vector.memset(negh_f, -0.5)
        negh = const.tile([P, 2], F32R, tag="negh", name="negh")
        nc.vector.tensor_copy(out=negh[:], in_=negh_f[:])
        ones_f = const.tile([1, N], F32, tag="ones_f", name="ones_f")
        nc.vector.memset(ones_f, 1.0)
        ones_r = const.tile([1, N], F32R, tag="ones_r", name="ones_r")
        nc.vector.tensor_copy(out=ones_r[:], in_=ones_f[:])

        # biases / rows / weight scales, packed in smA / smB
        smA = persist.tile([P, 102], F32, tag="smA", name="smA")
        nc.sync.dma_start(out=smA[:], in_=smA_g[:])
        # smA columns: qkbm 0-11 | qkbc 12-23 | fc1b 24-47 | wscales 48-101
        # (wscales: wqk +0..5 | wv +6..11 | wpm +12..17 | wpc +18..23 |
        #  wfc1 +24..29 | wfc2 +30..53)
        vb_b = persist.tile([P, D], F32, tag="vb_b", name="vb_b")
        nc.sync.dma_start(out=vb_b[:], in_=smB_d[0:1, :].to_broadcast([P, D]))
        r1m_b = persist.tile([P, D], F32, tag="r1m_b", name="r1m_b")
        nc.sync.dma_start(out=r1m_b[:], in_=smB_d[1:2, :].to_broadcast([P, D]))
        r1c_b = persist.tile([P, D], F32, tag="r1c_b", name="r1c_b")
        nc.sync.dma_start(out=r1c_b[:], in_=smB_d[2:3, :].to_broadcast([P, D]))
        r2_b = persist.tile([P, D], F32, tag="r2_b", name="r2_b")
        nc.sync.dma_start(out=r2_b[:], in_=smB_d[3:4, :].to_broadcast([P, D]))

        # int4 -> bf16 weight loads: unpack nibbles, scale per in-feature row
        def load_w4(pool, pw, dst, src_ap, scale_ap, wp):
            st = pool.tile([P, pw], U8, tag="wst", name="wst")
            lo = pool.tile([P, pw], U8, tag="wlo", name="wlo")
            hi = pool.tile([P, pw], U8, tag="whi", name="whi")
            nc.sync.dma_start(out=st[:, :wp], in_=src_ap)
            nc.vector.tensor_scalar(out=lo[:, :wp], in0=st[:, :wp], scalar1=15,
                                    scalar2=None, op0=ALU.bitwise_and)
            nc.vector.tensor_scalar(out=hi[:, :wp], in0=st[:, :wp], scalar1=4,
                                    scalar2=None, op0=ALU.logical_shift_right)
            dg = dst.rearrange("p (d t) -> p t d", t=2)
            nc.vector.tensor_scalar(out=dg[:, 0, :], in0=lo[:, :wp], scalar1=8.0,
                                    scalar2=scale_ap, op0=ALU.subtract, op1=ALU.mult)
            nc.vector.tensor_scalar(out=dg[:, 1, :], in0=hi[:, :wp], scalar1=8.0,
                                    scalar2=scale_ap, op0=ALU.subtract, op1=ALU.mult)

        # residual-stream tiles (bf16, natural layout); become x' in place.
        # Loaded as int8 + per-token scale; delta tiles d_t accumulate the
        # gamma-scaled branch sums (the device output is delta = y - x).
        xs_t = []
        for i, (n0, nn) in enumerate(NT):
            t = persist.tile([P, 2], F32, tag=f"xs{i}", name=f"xs{i}")
            nc.sync.dma_start(out=t[:nn, :], in_=xs_d[n0:n0 + nn, :])
            xs_t.append(t)
        x_t, d_t = {}, {}
        xqp = top.enter_context(tc.tile_pool(name="xq", bufs=3))
        for si, (s, src) in enumerate((("m", xm_d), ("c", xc_d))):
            for i, (n0, nn) in enumerate(NT):
                xq = xqp.tile([P, D // 2], U8, tag="xq", name="xq")
                nc.sync.dma_start(out=xq[:nn, :], in_=src[n0:n0 + nn, :])
                lo = xqp.tile([P, D // 2], U8, tag="xlo", name="xlo")
                hi = xqp.tile([P, D // 2], U8, tag="xhi", name="xhi")
                nc.vector.tensor_scalar(out=lo[:nn, :], in0=xq[:nn, :], scalar1=15,
                                        scalar2=None, op0=ALU.bitwise_and)
                nc.vector.tensor_scalar(out=hi[:nn, :], in0=xq[:nn, :], scalar1=4,
                                        scalar2=None, op0=ALU.logical_shift_right)
                t = persist.tile([P, D], BF16, tag=f"x_{s}{i}", name=f"x_{s}{i}")
                tg = t[:nn, :].rearrange("p (d t) -> p t d", t=2)
                sc_ap = xs_t[i][:nn, si:si + 1]
                nc.vector.tensor_scalar(out=tg[:, 0, :], in0=lo[:nn, :], scalar1=8.0,
                                        scalar2=sc_ap, op0=ALU.subtract, op1=ALU.mult)
                nc.vector.tensor_scalar(out=tg[:, 1, :], in0=hi[:nn, :], scalar1=8.0,
                                        scalar2=sc_ap, op0=ALU.subtract, op1=ALU.mult)
                x_t[s, i] = t
                d_t[s, i] = persist.tile([P, D], BF16, tag=f"d_{s}{i}", name=f"d_{s}{i}")

        # ---------- helpers ----------
        def layernorm_transpose(lnp, psln, s, xhatT):
            """LN over feature dim of x_t[s,*] then transpose into xhatT[j] tiles."""
            for i, (n0, nn) in enumerate(NT):
                xt = x_t[s, i]
                stats = lnp.tile([P, 3, 6], F32, tag="stats", name="stats")
                xg = xt[:nn, :].rearrange("p (g d) -> p g d", g=3)
                for g in range(3):
                    nc.vector.bn_stats(out=stats[:nn, g, :], in_=xg[:, g, :])
                mv = lnp.tile([P, 2], F32, tag="mv", name="mv")
                nc.vector.bn_aggr(out=mv[:nn], in_=stats[:nn])
                rstd = lnp.tile([P, 1], F32, tag="rstd", name="rstd")
                nc.scalar.activation(out=rstd[:nn], in_=mv[:nn, 1:2], func=AF.Sqrt,
                                     bias=eps_t[:nn], scale=1.0)
                nc.vector.reciprocal(out=rstd[:nn], in_=rstd[:nn])
                xhat = lnp.tile([P, D], BF16, tag="xhat", name="xhat")
                nc.vector.tensor_scalar(out=xhat[:nn], in0=xt[:nn, :],
                                        scalar1=mv[:nn, 0:1], scalar2=rstd[:nn],
                                        op0=ALU.subtract, op1=ALU.mult)
                for j in range(DT):
                    pst = psln.tile([P, P], BF16, tag="pst", name="pst")
                    nc.tensor.transpose(out=pst[:, :nn], in_=xhat[:nn, j * P:(j + 1) * P],
                                        identity=ident[:nn, :nn])
                    if j % 2 == 0:
                        nc.scalar.copy(out=xhatT[j][:, n0:n0 + nn], in_=pst[:, :nn])
                    else:
                        nc.vector.tensor_copy(out=xhatT[j][:, n0:n0 + nn], in_=pst[:, :nn])

        # ================= Phase A/B: LN1 + QKV =================
        # Pool lifetimes are a stack (LIFO release): ctx_io spans A/B..D and is
        # opened first; attn_io spans A/B..C and closes right after attention.
        ctx_cm = tc.tile_pool(name="ctx_io", bufs=1)
        ctx_io = ctx_cm.__enter__()
        ctxm = [ctx_io.tile([P, N], BF16, tag=f"ctxm{j}", name=f"ctxm{j}") for j in range(DT)]
        ctxc = [ctx_io.tile([P, N], BF16, tag=f"ctxc{j}", name=f"ctxc{j}") for j in range(DT)]
        attn_cm = tc.tile_pool(name="attn_io", bufs=1)
        attn_io = attn_cm.__enter__()
        qc = [attn_io.tile([P, N], BF16, tag=f"qc{h}", name=f"qc{h}") for h in range(H)]
        kc = [attn_io.tile([P, N], BF16, tag=f"kc{h}", name=f"kc{h}") for h in range(H)]
        vm = [attn_io.tile([P, H, HD + 1], BF16, tag=f"vm{i}", name=f"vm{i}") for i in range(5)]
        vc = [attn_io.tile([P, H, HD], BF16, tag=f"vc{i}", name=f"vc{i}") for i in range(5)]
        for i, (n0, nn) in enumerate(NT):
            nc.vector.memset(vm[i][:nn, :, HD:HD + 1], 1.0)

        with contextlib.ExitStack() as ab:
            wpool = ab.enter_context(tc.tile_pool(name="wqkv", bufs=1))
            wqk = [wpool.tile([P, 2 * D], BF16, tag=f"wqk{j}", name=f"wqk{j}") for j in range(DT)]
            wv = [wpool.tile([P, D], BF16, tag=f"wv{j}", name=f"wv{j}") for j in range(DT)]
            wstg1 = ab.enter_context(tc.tile_pool(name="wstg1", bufs=2))
            for j in range(DT):
                load_w4(wstg1, D, wqk[j][:], wqk_g[j * P:(j + 1) * P, :],
                        smA[:, 48 + j:49 + j], D)
                load_w4(wstg1, D, wv[j][:], wv_g[j * P:(j + 1) * P, :],
                        smA[:, 54 + j:55 + j], D // 2)

            xhatT = {s: [wpool.tile([P, N], BF16, tag=f"xhatT_{s}{j}", name=f"xhatT_{s}{j}") for j in range(DT)]
                     for s in ("m", "c")}
            lnp1 = ab.enter_context(tc.tile_pool(name="ln_ln1", bufs=3))
            psln1 = ab.enter_context(tc.tile_pool(name="psln_ln1", bufs=2, space="PSUM"))
            for s in ("m", "c"):
                layernorm_transpose(lnp1, psln1, s, xhatT[s])

            psqk = ab.enter_context(tc.tile_pool(name="psqk", bufs=3, space="PSUM"))
            sc1 = ab.enter_context(tc.tile_pool(name="sc_covqk", bufs=3))

            # --- QK GEMMs, transposed layout out [d_out, n] ---
            for s in ("m", "c"):
                for t in range(2 * DT):           # 6 q-tiles then 6 k-tiles
                    is_q = t < DT
                    for (c0, cw) in QCH:
                        ps = psqk.tile([P, 512], F32, tag="ps", name="ps")
                        for j in range(DT):
                            nc.tensor.matmul(ps[:, :cw], lhsT=wqk[j][:, t * P:(t + 1) * P],
                                             rhs=xhatT[s][j][:, c0:c0 + cw],
                                             start=(j == 0), stop=(j == DT - 1))
                        hpair = (t % DT) * 2      # heads 2*(t%6), +1
                        dst = qc if is_q else kc
                        if s == "m":
                            # mean stream: out = sc*(z + b)
                            sc = SCALE if is_q else 1.0
                            for half in range(2):
                                pr = slice(64 * half, 64 * half + 64)
                                nc.vector.tensor_scalar(
                                    out=dst[hpair + half][0:64, c0:c0 + cw],
                                    in0=ps[pr, :cw], scalar1=smA[pr, t:t + 1],
                                    scalar2=sc, op0=ALU.add, op1=ALU.mult)
                        else:
                            # cov stream: c = sqrt(elu(z + b) + 1)
                            t1 = sc1.tile([P, 512], F32, tag="t1", name="t1")
                            nc.vector.tensor_scalar_add(out=t1[:, :cw], in0=ps[:, :cw],
                                                        scalar1=smA[:, 12 + t:13 + t])
                            t2 = sc1.tile([P, 512], F32, tag="t2", name="t2")
                            nc.vector.tensor_scalar_min(out=t2[:, :cw], in0=t1[:, :cw], scalar1=0.0)
                            nc.scalar.activation(out=t2[:, :cw], in_=t2[:, :cw], func=AF.Exp)
                            nc.vector.scalar_tensor_tensor(out=t1[:, :cw], in0=t1[:, :cw],
                                                           scalar=0.0, in1=t2[:, :cw],
                                                           op0=ALU.max, op1=ALU.add)
                            for half in range(2):
                                pr = slice(64 * half, 64 * half + 64)
                                nc.scalar.activation(
                                    out=dst[hpair + half][64:128, c0:c0 + cw],
                                    in_=t1[pr, :cw], func=AF.Sqrt)

            # --- V GEMMs, natural layout out [n, d_v] ---
            for s in ("m", "c"):
                for i, (n0, nn) in enumerate(NT):
                    for c2, (v0, vw) in enumerate(VCH):
                        ps = psqk.tile([P, 512], F32, tag="ps", name="ps")
                        for j in range(DT):
                            nc.tensor.matmul(ps[:nn, :vw], lhsT=xhatT[s][j][:, n0:n0 + nn],
                                             rhs=wv[j][:, v0:v0 + vw],
                                             start=(j == 0), stop=(j == DT - 1))
                        psg = ps[:nn, :vw].rearrange("p (g d) -> p g d", g=6)
                        vbg = vb_b[:nn, v0:v0 + vw].rearrange("p (g d) -> p g d", g=6)
                        hs = slice(6 * c2, 6 * c2 + 6)
                        if s == "m":
                            nc.vector.tensor_tensor(out=vm[i][:nn, hs, 0:HD], in0=psg,
                                                    in1=vbg, op=ALU.add)
                        else:
                            t1 = sc1.tile([P, 512], F32, tag="t1", name="t1")
                            t1g = t1[:nn, :vw].rearrange("p (g d) -> p g d", g=6)
                            nc.vector.tensor_tensor(out=t1g, in0=psg, in1=vbg, op=ALU.add)
                            t2 = sc1.tile([P, 512], F32, tag="t2", name="t2")
                            nc.vector.tensor_scalar_min(out=t2[:nn, :vw], in0=t1[:nn, :vw],
                                                        scalar1=0.0)
                            nc.scalar.activation(out=t2[:nn, :vw], in_=t2[:nn, :vw], func=AF.Exp)
                            t2g = t2[:nn, :vw].rearrange("p (g d) -> p g d", g=6)
                            nc.vector.scalar_tensor_tensor(out=vc[i][:nn, hs, :], in0=t1g,
                                                           scalar=0.0, in1=t2g,
                                                           op0=ALU.max, op1=ALU.add)

        # ================= Phase C: attention =================
        with contextlib.ExitStack() as at:
            AB = at.enter_context(tc.tile_pool(name="AB", bufs=1))
            # per-head K=2 affine operands packed at 32-aligned partition slots
            # (base partition must be 0/32/64): head h -> tile h//3,
            # partitions (h%3)*32 + {0,1}. A = [colterm; ones], B = [ones; rowterm]
            N2 = N + 1   # fp32r needs even innermost extents; pad column never read
            A_pack = [AB.tile([P, N2], F32R, tag=f"A_pack{t}", name=f"A_pack{t}") for t in range(4)]
            B_pack = [AB.tile([P, N2], F32R, tag=f"B_pack{t}", name=f"B_pack{t}") for t in range(4)]

            def ab_slot(h):
                return A_pack[h // 3], B_pack[h // 3], (h % 3) * 32
            sqp = at.enter_context(tc.tile_pool(name="sqp", bufs=2))
            stg = at.enter_context(tc.tile_pool(name="stg", bufs=2))
            sigp = at.enter_context(tc.tile_pool(name="sigp", bufs=5))
            rpbp = at.enter_context(tc.tile_pool(name="rpbp", bufs=5))
            ep = at.enter_context(tc.tile_pool(name="ep", bufs=12))
            denp = at.enter_context(tc.tile_pool(name="denp", bufs=2))
            rcb = at.enter_context(tc.tile_pool(name="rcb", bufs=2))
            ps_r = at.enter_context(tc.tile_pool(name="ps_r", bufs=2, space="PSUM"))
            ps_s = at.enter_context(tc.tile_pool(name="ps_s", bufs=2, space="PSUM"))
            ps_c = at.enter_context(tc.tile_pool(name="ps_c", bufs=1, space="PSUM"))

            for h in range(H):
                # affine terms: A=[ -0.5*|w_k|^2 ; 1 ], B=[ 1 ; -0.5*|u_q|^2 ]
                A_t, B_t, sl = ab_slot(h)
                nc.sync.dma_start(out=A_t[sl + 1:sl + 2, :N], in_=ones_r[:])
                nc.vector.tensor_copy(out=B_t[sl:sl + 1, :N], in_=ones_r[:])
                sq_k = sqp.tile([P, N2], F32R, tag="sq", name="sq")
                nc.vector.tensor_tensor(out=sq_k[:, :N], in0=kc[h][:], in1=kc[h][:], op=ALU.mult)
                for (c0, cw) in QCH:
                    cwe = cw + (cw % 2)
                    pr = ps_r.tile([2, 512], F32, tag="pr", name="pr")
                    nc.tensor.matmul(pr[:, :cwe], lhsT=negh[:], rhs=sq_k[:, c0:c0 + cwe],
                                     start=True, stop=True)
                    nc.scalar.copy(out=A_t[sl:sl + 1, c0:c0 + cw], in_=pr[0:1, :cw])
                sq_q = sqp.tile([P, N2], F32R, tag="sq", name="sq")
                nc.vector.tensor_tensor(out=sq_q[:, :N], in0=qc[h][:], in1=qc[h][:], op=ALU.mult)
                rowst = stg.tile([1, N], F32R, tag="rowst", name="rowst")
                for (c0, cw) in QCH:
                    cwe = cw + (cw % 2)
                    pr = ps_r.tile([2, 512], F32, tag="pr", name="pr")
                    nc.tensor.matmul(pr[:, :cwe], lhsT=negh[:], rhs=sq_q[:, c0:c0 + cwe],
                                     start=True, stop=True)
                    nc.scalar.copy(out=rowst[0:1, c0:c0 + cw], in_=pr[0:1, :cw])
                nc.sync.dma_start(out=B_t[sl + 1:sl + 2, :N], in_=rowst[:])

                # scores + sigmoid + rpb + exp, S_T layout [k, q]
                e_h, e2_h = [], []
                for kt, (k0, kn) in enumerate(NT):
                    r0 = h * N + k0
                    rpq = rpbp.tile([P, NP2], U8, tag="rpb", name="rpb")
                    nc.sync.dma_start(out=rpq[:kn, :], in_=rpb_g[r0:r0 + kn, :])
                    rps = rpbp.tile([P, 1], F32, tag="rps", name="rps")
                    nc.sync.dma_start(out=rps[:kn, :], in_=rpbs_g[r0:r0 + kn, :])
                    rlo = rpbp.tile([P, NP2], U8, tag="rlo", name="rlo")
                    rhi = rpbp.tile([P, NP2], U8, tag="rhi", name="rhi")
                    nc.vector.tensor_scalar(out=rlo[:kn, :], in0=rpq[:kn, :], scalar1=15,
                                            scalar2=None, op0=ALU.bitwise_and)
                    nc.vector.tensor_scalar(out=rhi[:kn, :], in0=rpq[:kn, :], scalar1=4,
                                            scalar2=None, op0=ALU.logical_shift_right)
                    rpb_b = rpbp.tile([P, 2 * NP2], BF16, tag="rpbb", name="rpbb")
                    rg = rpb_b[:kn, :].rearrange("p (d t) -> p t d", t=2)
                    nc.vector.tensor_scalar_mul(out=rg[:, 0, :], in0=rlo[:kn, :],
                                                scalar1=rps[:kn, 0:1])
                    nc.vector.tensor_scalar_mul(out=rg[:, 1, :], in0=rhi[:kn, :],
                                                scalar1=rps[:kn, 0:1])
                    sig = sigp.tile([P, N], F32, tag="sig", name="sig")
                    e_t = ep.tile([P, N], BF16, tag="e", name="e")
                    e2_t = ep.tile([P, N], BF16, tag="e2", name="e2")
                    for (c0, cw) in QCH:
                        ps = ps_s.tile([P, 512], F32, tag="ps", name="ps")
                        A_t, B_t, sl = ab_slot(h)
                        kne = kn + (kn % 2)
                        cwe = cw + (cw % 2)
                        nc.tensor.matmul(ps[:kn, :cw], lhsT=kc[h][:, k0:k0 + kn],
                                         rhs=qc[h][:, c0:c0 + cw], start=True, stop=False)
                        nc.tensor.matmul(ps[:kne, :cwe], lhsT=A_t[sl:sl + 2, k0:k0 + kne],
                                         rhs=B_t[sl:sl + 2, c0:c0 + cwe], start=False, stop=True,
                                         skip_group_check=True)
                        # sigmoid(2x) = 0.5*tanh(x) + 0.5; tanh shares the ACT
                        # table set with exp.
                        nc.scalar.activation(out=sig[:kn, c0:c0 + cw], in_=ps[:kn, :cw],
                                             func=AF.Tanh, scale=1.0)
                    # e = exp(0.5*tanh + 0.5) * exp(rpb)   (rpb sent as fp8 factor)
                    nc.scalar.activation(out=sig[:kn, :], in_=sig[:kn, :], func=AF.Exp,
                                         bias=half_t[:kn], scale=0.5)
                    nc.vector.tensor_tensor(out=e_t[:kn, :], in0=sig[:kn, :],
                                            in1=rpb_b[:kn, :N], op=ALU.mult)
                    nc.gpsimd.tensor_tensor(out=e2_t[:kn, :], in0=e_t[:kn, :],
                                            in1=e_t[:kn, :], op=ALU.mult)
                    e_h.append(e_t)
                    e2_h.append(e2_t)

                # context matmuls (unnormalized) + per-chunk denominator:
                # each chunk's reciprocal/broadcast/evict chain depends only on
                # its own denominator slice, so chunks (and heads) pipeline.
                den = denp.tile([1, N], F32, tag="den", name="den")
                recip = denp.tile([1, N], F32, tag="recip", name="recip")
                rb = rcb.tile([64, N], F32, tag="rb", name="rb")
                rb2 = rcb.tile([64, N], F32, tag="rb2", name="rb2")
                jt, rr = h // 2, slice(64 * (h % 2), 64 * (h % 2) + 64)
                for ci, (c0, cw) in enumerate(QCH):
                    pm = ps_c.tile([65, 512], F32, tag=f"pcm{ci}", name=f"pcm{ci}")
                    pc2 = ps_c.tile([64, 512], F32, tag=f"pcc{ci}", name=f"pcc{ci}")
                    for kt, (k0, kn) in enumerate(NT):
                        nc.tensor.matmul(pm[:, :cw], lhsT=vm[kt][:kn, h, :],
                                         rhs=e_h[kt][:kn, c0:c0 + cw],
                                         start=(kt == 0), stop=(kt == 4))
                        nc.tensor.matmul(pc2[:, :cw], lhsT=vc[kt][:kn, h, :],
                                         rhs=e2_h[kt][:kn, c0:c0 + cw],
                                         start=(kt == 0), stop=(kt == 4))
                    nc.scalar.copy(out=den[0:1, c0:c0 + cw], in_=pm[64:65, :cw])
                    nc.vector.reciprocal(out=recip[0:1, c0:c0 + cw],
                                         in_=den[0:1, c0:c0 + cw])
                    nc.gpsimd.partition_broadcast(rb[:, c0:c0 + cw],
                                                  recip[0:1, c0:c0 + cw])
                    nc.vector.tensor_tensor(out=rb2[:, c0:c0 + cw],
                                            in0=rb[:, c0:c0 + cw],
                                            in1=rb[:, c0:c0 + cw], op=ALU.mult)
                    nc.vector.tensor_tensor(out=ctxm[jt][rr, c0:c0 + cw],
                                            in0=pm[0:64, :cw],
                                            in1=rb[:, c0:c0 + cw], op=ALU.mult)
                    nc.vector.tensor_tensor(out=ctxc[jt][rr, c0:c0 + cw],
                                            in0=pc2[0:64, :cw],
                                            in1=rb2[:, c0:c0 + cw], op=ALU.mult)

        attn_cm.__exit__(None, None, None)

        # ================= Phase D: proj + residual =================
        with contextlib.ExitStack() as pd:
            wpp = pd.enter_context(tc.tile_pool(name="wproj", bufs=1))
            wpm = [wpp.tile([P, D], BF16, tag=f"wpm{j}", name=f"wpm{j}") for j in range(DT)]
            wpc = [wpp.tile([P, D], BF16, tag=f"wpc{j}", name=f"wpc{j}") for j in range(DT)]
            wstg2 = pd.enter_context(tc.tile_pool(name="wstg2", bufs=2))
            for j in range(DT):
                load_w4(wstg2, D // 2, wpm[j][:], wpm_g[j * P:(j + 1) * P, :],
                        smA[:, 60 + j:61 + j], D // 2)
                load_w4(wstg2, D // 2, wpc[j][:], wpc_g[j * P:(j + 1) * P, :],
                        smA[:, 66 + j:67 + j], D // 2)
            psp = pd.enter_context(tc.tile_pool(name="psproj", bufs=3, space="PSUM"))
            for s, ctx_t, wp, rb_row in (("m", ctxm, wpm, r1m_b), ("c", ctxc, wpc, r1c_b)):
                for i, (n0, nn) in enumerate(NT):
                    for (v0, vw) in VCH:
                        ps = psp.tile([P, 512], F32, tag="ps", name="ps")
                        for j in range(DT):
                            nc.tensor.matmul(ps[:nn, :vw], lhsT=ctx_t[j][:, n0:n0 + nn],
                                             rhs=wp[j][:, v0:v0 + vw],
                                             start=(j == 0), stop=(j == DT - 1))
                        xt, dt = x_t[s, i], d_t[s, i]
                        # dd = gamma1*proj(ctx) + r1  (gamma folded into w-scales)
                        nc.vector.tensor_tensor(out=dt[:nn, v0:v0 + vw],
                                                in0=ps[:nn, :vw],
                                                in1=rb_row[:nn, v0:v0 + vw], op=ALU.add)
                        nc.vector.tensor_tensor(out=xt[:nn, v0:v0 + vw],
                                                in0=xt[:nn, v0:v0 + vw],
                                                in1=dt[:nn, v0:v0 + vw], op=ALU.add)

        ctx_cm.__exit__(None, None, None)

        # ================= Phase E/F: LN2 + MLP =================
        with contextlib.ExitStack() as pf:
            wfp = pf.enter_context(tc.tile_pool(name="wfc", bufs=1))
            wfc1 = [wfp.tile([P, DFF], BF16, tag=f"wfc1_{j}", name=f"wfc1_{j}") for j in range(DT)]
            wstg3 = pf.enter_context(tc.tile_pool(name="wstg3", bufs=2))
            for j in range(DT):
                load_w4(wstg3, DFF // 2, wfc1[j][:], wfc1_g[j * P:(j + 1) * P, :],
                        smA[:, 72 + j:73 + j], DFF // 2)
            wfc2 = [wfp.tile([P, D], BF16, tag=f"wfc2_{f}", name=f"wfc2_{f}") for f in range(FT)]
            for f in range(FT):
                load_w4(wstg3, DFF // 2, wfc2[f][:], wfc2_g[f * P:(f + 1) * P, :],
                        smA[:, 78 + f:79 + f], D // 2)

            xhat2T = {s: [wfp.tile([P, N], BF16, tag=f"xh2T_{s}{j}", name=f"xh2T_{s}{j}") for j in range(DT)]
                      for s in ("m", "c")}
            lnp2 = pf.enter_context(tc.tile_pool(name="ln_ln2", bufs=3))
            psln2 = pf.enter_context(tc.tile_pool(name="psln_ln2", bufs=2, space="PSUM"))
            for s in ("m", "c"):
                layernorm_transpose(lnp2, psln2, s, xhat2T[s])

            psf = pf.enter_context(tc.tile_pool(name="psfc", bufs=4, space="PSUM"))
            hp = pf.enter_context(tc.tile_pool(name="hT", bufs=1))
            outp = pf.enter_context(tc.tile_pool(name="outp", bufs=3))
            for si, s in enumerate(("m", "c")):
                # hT tiles shared between streams (tag reuse serializes via deps)
                hT = {s: [hp.tile([P, N], BF16, tag=f"hT{f}", name=f"hT{f}")
                          for f in range(FT)]}
                for f in range(FT):
                    for (c0, cw) in QCH:
                        ps = psf.tile([P, 512], F32, tag="ps", name="ps")
                        for j in range(DT):
                            nc.tensor.matmul(ps[:, :cw], lhsT=wfc1[j][:, f * P:(f + 1) * P],
                                             rhs=xhat2T[s][j][:, c0:c0 + cw],
                                             start=(j == 0), stop=(j == DT - 1))
                        nc.scalar.activation(out=hT[s][f][:, c0:c0 + cw], in_=ps[:, :cw],
                                             func=AF.Gelu, bias=smA[:, 24 + f:25 + f],
                                             scale=1.0)
                for i, (n0, nn) in enumerate(NT):
                    # delta = d_attn + gamma2*mlp(...) + r2; quantize per token
                    yt = outp.tile([P, D], F32, tag="yt", name="yt")
                    yq = outp.tile([P, D], U8, tag="yq", name="yq")
                    for (v0, vw) in VCH:
                        ps = psf.tile([P, 512], F32, tag="ps", name="ps")
                        for f in range(FT):
                            nc.tensor.matmul(ps[:nn, :vw], lhsT=hT[s][f][:, n0:n0 + nn],
                                             rhs=wfc2[f][:, v0:v0 + vw],
                                             start=(f == 0), stop=(f == FT - 1))
                        # gamma2*fc2(h) + d_attn  (gamma folded into w-scales)
                        nc.vector.tensor_tensor(out=yt[:nn, v0:v0 + vw], in0=ps[:nn, :vw],
                                                in1=d_t[s, i][:nn, v0:v0 + vw], op=ALU.add)
                        nc.vector.tensor_tensor(out=yt[:nn, v0:v0 + vw],
                                                in0=yt[:nn, v0:v0 + vw],
                                                in1=r2_b[:nn, v0:v0 + vw], op=ALU.add)
                    am = outp.tile([P, 1], F32, tag="am", name="am")
                    nc.vector.tensor_reduce(out=am[:nn], in_=yt[:nn, :], axis=AXL.X,
                                            op=ALU.max, apply_absolute_value=True)
                    sc_t = outp.tile([P, 1], F32, tag="sc", name="sc")
                    nc.vector.tensor_scalar(out=sc_t[:nn], in0=am[:nn],
                                            scalar1=1.0 / 7.0, scalar2=1e-30,
                                            op0=ALU.mult, op1=ALU.max)
                    inv = outp.tile([P, 1], F32, tag="inv", name="inv")
                    nc.vector.reciprocal(out=inv[:nn], in_=sc_t[:nn])
                    # biased nibble q+8 in [1,15], then pack pairs: lo|hi<<4
                    nc.vector.tensor_scalar(out=yq[:nn, :], in0=yt[:nn, :],
                                            scalar1=inv[:nn], scalar2=8.0,
                                            op0=ALU.mult, op1=ALU.add)
                    qg = yq[:nn, :].rearrange("p (d t) -> p t d", t=2)
                    pk = outp.tile([P, D // 2], U8, tag="pk", name="pk")
                    nc.vector.scalar_tensor_tensor(out=pk[:nn, :], in0=qg[:, 1, :],
                                                   scalar=16.0, in1=qg[:, 0, :],
                                                   op0=ALU.mult, op1=ALU.add)
                    nc.sync.dma_start(out=y_d[si * N + n0:si * N + n0 + nn, :],
                                      in_=pk[:nn, :])
                    nc.sync.dma_start(out=ysc_d[n0:n0 + nn, si:si + 1], in_=sc_t[:nn])

    nc.compile()
    return nc


def _prep_shared(inputs):
    f32 = np.float32
    g = lambda k: np.asarray(inputs[k], f32)
    qkv_w, norm1_w, norm1_b = g("qkv_w"), g("norm1_w"), g("norm1_b")
    qkv_w_eff = qkv_w * norm1_w[None, :]
    qkv_b_eff = qkv_w_eff @ norm1_b

    wqkT = np.ascontiguousarray(qkv_w_eff[:2 * D].T)
    wvT = np.ascontiguousarray(qkv_w_eff[2 * D:].T)
    qkb = qkv_b_eff[:2 * D]
    vb = qkv_b_eff[2 * D:]

    gamma1, gamma2 = g("gamma1"), g("gamma2")
    proj_w, proj_b = g("proj_w"), g("proj_b")
    cov_proj_w, cov_proj_b = g("cov_proj_w"), g("cov_proj_b")
    norm2_w, norm2_b = g("norm2_w"), g("norm2_b")
    fc1_w, fc1_b = g("fc1_w"), g("fc1_b")
    fc2_w, fc2_b = g("fc2_w"), g("fc2_b")

    fc1_w_eff = fc1_w * norm2_w[None, :]
    fc1_b_eff = fc1_b + fc1_w_eff @ norm2_b

    def pack_w(Wt):
        """Per-row symmetric int4, nibble-packed; returns (u8 [r, c/2], s [r])."""
        amax = np.abs(Wt).max(1)
        s = np.maximum(amax, 1e-30) / 7.0
        q8 = (np.rint(Wt / s[:, None]) + 8.0).astype(np.uint8)
        return q8[:, 0::2] | (q8[:, 1::2] << 4), s.astype(f32)

    p_qk, s_qk = pack_w(wqkT)
    p_v, s_v = pack_w(wvT)
    p_pm, s_pm = pack_w(np.ascontiguousarray((gamma1[:, None] * proj_w).T))
    p_pc, s_pc = pack_w(np.ascontiguousarray((gamma1[:, None] * cov_proj_w).T))
    p_f1, s_f1 = pack_w(np.ascontiguousarray(fc1_w_eff.T))
    p_f2, s_f2 = pack_w(np.ascontiguousarray((gamma2[:, None] * fc2_w).T))

    # rel_pos_bias as multiplicative exp(rpb), int4 unsigned w/ per-row scale
    NP2 = (N + 1) // 2
    rpbT = np.ascontiguousarray(
        np.asarray(inputs["rel_pos_bias"], f32)[0].transpose(0, 2, 1))
    Rp = np.zeros((RPB_PAD, 2 * NP2), f32)
    Rp[:RPB_ROWS, :N] = np.exp(rpbT.reshape(RPB_ROWS, N))
    rs = np.maximum(Rp.max(1), 1e-30) / 15.0
    Rq = np.rint(Rp / rs[:, None]).astype(np.uint8)
    rpb_pk = Rq[:, 0::2] | (Rq[:, 1::2] << 4)

    ws = np.concatenate([s_qk, s_v, s_pm, s_pc, s_f1, s_f2])   # 6912
    smA = np.concatenate([
        np.ascontiguousarray(qkb.reshape(12, P).T),
        np.ascontiguousarray(qkb.reshape(12, P).T),
        np.ascontiguousarray(fc1_b_eff.reshape(FT, P).T),
        np.ascontiguousarray(ws.reshape(54, P).T),
    ], axis=1).astype(f32)
    smB = np.stack([vb, gamma1 * proj_b, gamma1 * cov_proj_b,
                    gamma2 * fc2_b]).astype(f32)

    shared = {"smB": smB}
    full = {
        "smA_sh": smA,
        "wqk_sh": p_qk,
        "wv_sh": p_v,
        "rpb_sh": rpb_pk,
        "rpbs_sh": rs.astype(f32).reshape(RPB_PAD, 1),
        "wpm_sh": p_pm,
        "wpc_sh": p_pc,
        "wfc1_sh": p_f1,
        "wfc2_sh": p_f2,
    }
    shards = {}
    for k, v in full.items():
        ch = v.shape[0] // NC
        shards[k] = [np.ascontiguousarray(v[c * ch:(c + 1) * ch]) for c in range(NC)]
    return shared, shards


def _quant_rows4(x):
    """Per-token symmetric int4 packed 2/byte: (p [N,D/2] u8, s [N] f32)."""
    amax = np.abs(x).max(axis=1)
    s = np.maximum(amax, 1e-30) / 7.0
    q8 = (np.rint(x / s[:, None]) + 8.0).astype(np.uint8)   # [1,15]
    p = q8[:, 0::2] | (q8[:, 1::2] << 4)
    return p, s.astype(np.float32)


def _prepare(inputs):
    """Build per-core input maps (host-side prep, outside the hot path)."""
    shared, shards = _prep_shared(inputs)
    xm = np.asarray(inputs["x_mean"], np.float32)
    xc = np.asarray(inputs["x_cov"], np.float32)

    in_maps = []
    for b in range(B):
        m = dict(shared)
        for k, lst in shards.items():
            m[k] = lst[b]
        qm, sm = _quant_rows4(xm[b])
        qc_, sc = _quant_rows4(xc[b])
        m["xm"] = qm
        m["xc"] = qc_
        m["xs"] = np.stack([sm, sc], 1)
        m["_xf32"] = (xm[b], xc[b])   # host-side residual base (not a param)
        in_maps.append(m)
    return in_maps


def _make_runner(nc, n_cores):
    """Cached-jit SPMD dispatcher (mirrors bass2jax.run_bass_via_pjrt, but the
    jit is built once, and the donated zero output buffers are created
    on-device instead of being shipped from the host every call)."""
    bass2jax.install_neuronx_cc_hook()
    assert nc.dbg_addr is None

    partition_name = nc.partition_id_tensor.name if nc.partition_id_tensor else None
    in_names, out_names, out_avals, in_shapes = [], [], [], []
    for alloc in nc.m.functions[0].allocations:
        if not isinstance(alloc, mybir.MemoryLocationSet):
            continue
        name = alloc.memorylocations[0].name
        if alloc.kind == "ExternalInput":
            if name != partition_name:
                in_names.append(name)
                in_shapes.append((tuple(alloc.tensor_shape), mybir.dt.np(alloc.dtype)))
        elif alloc.kind == "ExternalOutput":
            shape = tuple(alloc.tensor_shape)
            dtype = mybir.dt.np(alloc.dtype)
            out_names.append(name)
            out_avals.append(jax.core.ShapedArray(shape, dtype))
    n_params = len(in_names)
    n_outs = len(out_names)
    param_names = list(in_names)
    in_names = in_names + out_names
    if partition_name is not None:
        in_names.append(partition_name)

    devices = jax.devices()[:n_cores]
    assert len(devices) == n_cores
    mesh = Mesh(np.asarray(devices), ("core",))
    donate = tuple(range(n_params, n_params + n_outs))

    def _body(*args):
        operands = list(args)
        if partition_name is not None:
            operands.append(bass2jax.partition_id_tensor())
        outs = bass2jax._bass_exec_p.bind(
            *operands,
            out_avals=tuple(out_avals),
            in_names=tuple(in_names),
            out_names=tuple(out_names),
            lowering_input_output_aliases=(),
            sim_require_finite=True,
            sim_require_nnan=True,
            nc=nc,
        )
        return tuple(outs)

    sm = shard_map(_body, mesh=mesh,
                   in_specs=(PartitionSpec("core"),) * (n_params + n_outs),
                   out_specs=(PartitionSpec("core"),) * n_outs,
                   check_rep=False)
    try:
        # AOT compile with bass_effect suppressed -> C++ fast-path dispatch
        example = [jax.ShapeDtypeStruct((n_cores * s[0], *s[1:]), dt)
                   for s, dt in in_shapes]
        example += [jax.ShapeDtypeStruct((n_cores * a.shape[0], *a.shape[1:]), a.dtype)
                    for a in out_avals]
        sharded = bass2jax.fast_dispatch_compile(
            lambda: jax.jit(sm, donate_argnums=donate, keep_unused=True)
            .lower(*example).compile())
    except Exception:
        sharded = jax.jit(sm, donate_argnums=donate, keep_unused=True)

    zero_shardings = tuple(NamedSharding(mesh, PartitionSpec("core"))
                           for _ in range(n_outs))

    def _zeros():
        return tuple(jnp.zeros((n_cores * a.shape[0], *a.shape[1:]), a.dtype)
                     for a in out_avals)

    zeros_jit = jax.jit(_zeros, out_shardings=zero_shardings)

    return {
        "sharded": sharded,
        "zeros_jit": zeros_jit,
        "param_names": param_names,
        "out_names": out_names,
        "out_avals": out_avals,
        "n_cores": n_cores,
    }


def _execute(in_maps):
    """One full dispatch: H2D, 8-core SPMD NEFF execution (with on-device
    AllGather of the sharded tensors), D2H, output assembly.

    The donated output buffers are recycled from the previous call (the
    kernel writes every output element, so their contents are irrelevant);
    only the first call pays for an on-device zeros launch. D2H pulls the
    8 per-device shards concurrently."""
    r = _CACHE["runner"]
    n_cores = r["n_cores"]
    per_core = [[np.asarray(m[name]) for name in r["param_names"]] for m in in_maps]
    concat_in = [
        np.concatenate([per_core[c][i] for c in range(n_cores)], axis=0)
        for i in range(len(r["param_names"]))
    ]
    don = _CACHE.pop("donate_bufs", None)
    if don is None:
        don = r["zeros_jit"]()
    out_arrs = r["sharded"](*concat_in, *don)
    _CACHE["donate_bufs"] = out_arrs

    pool = _CACHE.setdefault("d2h_pool", ThreadPoolExecutor(3 * n_cores))
    shards = []
    for o in out_arrs:
        byidx = sorted(o.addressable_shards, key=lambda s: s.index[0].start or 0)
        shards.extend(byidx)
    parts = list(pool.map(lambda s: np.asarray(s.data), shards))
    res = {}
    for i, name in enumerate(r["out_names"]):
        res[name] = np.stack(parts[i * n_cores:(i + 1) * n_cores], 0)
    # reconstruct y = x_f32 + dequant(unpack_int4(delta)), threaded per core
    ysc = res["ysc"].astype(np.float32)            # [B, N, 2]
    ym = np.empty((B, N, D), np.float32)
    yc = np.empty((B, N, D), np.float32)

    def rebuild(task):
        b, dst, xf, col = task
        p = res["y"][b][col * N:(col + 1) * N]
        q = np.empty((N, D), np.float32)
        q[:, 0::2] = (p & 15).astype(np.float32)
        q[:, 1::2] = (p >> 4).astype(np.float32)
        dst[b] = xf + (q - 8.0) * ysc[b, :, col:col + 1]

    list(pool.map(rebuild, [(b, dst, in_maps[b]["_xf32"][col], col)
                            for b in range(B)
                            for dst, col in ((ym, 0), (yc, 1))]))
    return ym, yc


def kernel(**inputs):
    if "nc" not in _CACHE:
        _CACHE["nc"] = _build_program()
        _CACHE["runner"] = _make_runner(_CACHE["nc"], NC)
    return _execute(_prepare(inputs))


# revision 64
# speedup vs baseline: 1.1529x; 1.0600x over previous
"""Trainium2 Bass kernel for the Wasserstein-attention transformer block.

Strategy: data-parallel over batch B=8 across 8 NeuronCores (one batch
element per core). Per core, the whole block runs with activations kept
in a transposed [feature, token] layout so every GEMM contracts over
partitions without runtime transposes of large tensors; attention runs
in S_T = [key, query] layout so softmax denominators and context
accumulation are plain matmuls. Matmul operands are bf16 (PSUM
accumulation fp32); the Wasserstein affine terms use f32r.

Host<->device traffic (the axon tunnel is ~35-65 MB/s and dominates the
wall clock) is minimized:
 - all shared tensors (weights + exp(rel_pos_bias)) are sent SHARDED 1/8
   per core and AllGather-ed on-device over the NeuronLink fabric;
 - weights and exp(rpb) travel as int4 (two nibbles per byte) with
   per-row f32 scales (gamma factors folded into the scales);
 - x inputs travel as int4 with per-token scales; the device returns
   only delta = y - x as per-token int4, and the host reconstructs
   y = x_f32 + dequant(delta), so the residual-stream precision never
   depends on the wire precision;
 - dispatch goes through a cached jit (no per-call retrace), donated
   output buffers are recycled from the previous call, and D2H pulls
   all shards concurrently.
"""
import contextlib
from concurrent.futures import ThreadPoolExecutor

import numpy as np
import ml_dtypes

import jax
import jax.numpy as jnp
from jax.experimental.shard_map import shard_map
from jax.sharding import Mesh, NamedSharding, PartitionSpec

import concourse.bass as bass
import concourse.tile as tile
from concourse import bacc, bass2jax, mybir
from concourse.masks import make_identity

F32 = mybir.dt.float32
F32R = mybir.dt.float32r
BF16 = mybir.dt.bfloat16
I8 = mybir.dt.int8
U8 = mybir.dt.uint8
F8 = mybir.dt.float8e4
AF = mybir.ActivationFunctionType
ALU = mybir.AluOpType
AXL = mybir.AxisListType

B, N, D, H = 8, 577, 768, 12
HD = D // H
DFF = 4 * D
SCALE = HD ** -0.5
LN_EPS = 1e-5

P = 128
NT = [(0, 128), (128, 128), (256, 128), (384, 128), (512, 65)]   # token tiles
QCH = [(0, 290), (290, 287)]                                     # psum-free chunks of N (both f32r-fast)
DT = D // P        # 6
FT = DFF // P      # 24
VCH = [(0, 384), (384, 384)]                                     # v / proj / fc2 out chunks

NC = 8             # cores
DSH = D // NC      # 96   row-shard of [D, *] weights
FSH = DFF // NC    # 384  row-shard of [DFF, *] weights
RPB_ROWS = H * N   # 6924
RPB_PAD = ((RPB_ROWS + NC - 1) // NC) * NC   # 6928
RPB_SH = RPB_PAD // NC                       # 866

GROUPS = [list(range(NC))]

_CACHE = {}


def _build_program():
    nc = bacc.Bacc("TRN2", target_bir_lowering=False, debug=False, num_devices=NC)

    # ---- DRAM I/O ----
    # per-core unique: int4 per-token-quantized x packed two-per-byte
    # (even col low nibble), scales in xs (col0=m, col1=c)
    xm_d = nc.declare_dram_parameter("xm", [N, D // 2], U8, isOutput=False)
    xc_d = nc.declare_dram_parameter("xc", [N, D // 2], U8, isOutput=False)
    xs_d = nc.declare_dram_parameter("xs", [N, 2], F32, isOutput=False)
    # sharded 1/8 per core, AllGather-ed on device; weights and exp(rpb)
    # travel as int4 packed two-per-byte with per-row f32 scales
    wqk_sh_d = nc.declare_dram_parameter("wqk_sh", [DSH, D], U8, isOutput=False)
    wv_sh_d = nc.declare_dram_parameter("wv_sh", [DSH, D // 2], U8, isOutput=False)
    rpb_sh_d = nc.declare_dram_parameter("rpb_sh", [RPB_SH, (N + 1) // 2], U8, isOutput=False)
    rpbs_sh_d = nc.declare_dram_parameter("rpbs_sh", [RPB_SH, 1], F32, isOutput=False)
    wpm_sh_d = nc.declare_dram_parameter("wpm_sh", [DSH, D // 2], U8, isOutput=False)
    wpc_sh_d = nc.declare_dram_parameter("wpc_sh", [DSH, D // 2], U8, isOutput=False)
    wfc1_sh_d = nc.declare_dram_parameter("wfc1_sh", [DSH, DFF // 2], U8, isOutput=False)
    wfc2_sh_d = nc.declare_dram_parameter("wfc2_sh", [FSH, D // 2], U8, isOutput=False)
    # smA = qkbm(12) | qkbc(12) | fc1b(24) | wscales(54), row-sharded 1/8
    # smB rows: vb, r1m, r1c, r2 (replicated, tiny)
    smA_sh_d = nc.declare_dram_parameter("smA_sh", [P // NC, 102], F32, isOutput=False)
    smB_d = nc.declare_dram_parameter("smB", [4, D], F32, isOutput=False)
    # outputs: int4 per-token-quantized delta (y - x), two nibbles per byte
    # (even col in low nibble, odd in high); y rows 0..N-1 = mean stream,
    # N..2N-1 = cov stream; scales in ysc
    y_d = nc.declare_dram_parameter("y", [2 * N, D // 2], U8, isOutput=True)
    ysc_d = nc.declare_dram_parameter("ysc", [N, 2], F32, isOutput=True)

    with tile.TileContext(nc) as tc, contextlib.ExitStack() as top:
        # ---- on-device AllGather of the sharded shared tensors ----
        dram = top.enter_context(tc.tile_pool(name="dram_cc", bufs=1, space="DRAM"))

        def gathered(param, chunk, full, tag, dt=BF16):
            bt = dram.tile(list(chunk), dt, tag=f"b_{tag}", name=f"b_{tag}")
            gt = dram.tile(list(full), dt, tag=f"g_{tag}", name=f"g_{tag}")
            nc.sync.dma_start(out=bt[:], in_=param[:])
            nc.gpsimd.collective_compute(
                "AllGather", ALU.bypass, replica_groups=GROUPS,
                ins=[bt.opt()], outs=[gt.opt()])
            return gt

        # ordered by first use: qkv -> rpb -> proj -> mlp
        NP2 = (N + 1) // 2
        wqk_g = gathered(wqk_sh_d, (DSH, D), (D, D), "wqk", dt=U8)
        wv_g = gathered(wv_sh_d, (DSH, D // 2), (D, D // 2), "wv", dt=U8)
        rpb_g = gathered(rpb_sh_d, (RPB_SH, NP2), (RPB_PAD, NP2), "rpb", dt=U8)
        rpbs_g = gathered(rpbs_sh_d, (RPB_SH, 1), (RPB_PAD, 1), "rpbs", dt=F32)
        wpm_g = gathered(wpm_sh_d, (DSH, D // 2), (D, D // 2), "wpm", dt=U8)
        wpc_g = gathered(wpc_sh_d, (DSH, D // 2), (D, D // 2), "wpc", dt=U8)
        wfc1_g = gathered(wfc1_sh_d, (DSH, DFF // 2), (D, DFF // 2), "wfc1", dt=U8)
        wfc2_g = gathered(wfc2_sh_d, (FSH, D // 2), (DFF, D // 2), "wfc2", dt=U8)
        smA_g = gathered(smA_sh_d, (P // NC, 102), (P, 102), "smA", dt=F32)

        const = top.enter_context(tc.tile_pool(name="const", bufs=1))
        persist = top.enter_context(tc.tile_pool(name="persist", bufs=1))

        ident = const.tile([P, P], BF16, tag="ident", name="ident")
        make_identity(nc, ident)
        eps_t = const.tile([P, 1], F32, tag="eps", name="eps")
        nc.vector.memset(eps_t, LN_EPS)
        half_t = const.tile([P, 1], F32, tag="half", name="half")
        nc.vector.memset(half_t, 0.5)
        negh_f = const.tile([P, 2], F32, tag="negh_f", name="negh_f")
        nc.vector.memset(negh_f, -0.5)
        negh = const.tile([P, 2], F32R, tag="negh", name="negh")
        nc.vector.tensor_copy(out=negh[:], in_=negh_f[:])
        ones_f = const.tile([1, N], F32, tag="ones_f", name="ones_f")
        nc.vector.memset(ones_f, 1.0)
        ones_r = const.tile([1, N], F32R, tag="ones_r", name="ones_r")
        nc.vector.tensor_copy(out=ones_r[:], in_=ones_f[:])

        # biases / rows / weight scales, packed in smA / smB
        smA = persist.tile([P, 102], F32, tag="smA", name="smA")
        nc.sync.dma_start(out=smA[:], in_=smA_g[:])
        # smA columns: qkbm 0-11 | qkbc 12-23 | fc1b 24-47 | wscales 48-101
        # (wscales: wqk +0..5 | wv +6..11 | wpm +12..17 | wpc +18..23 |
        #  wfc1 +24..29 | wfc2 +30..53)
        vb_b = persist.tile([P, D], F32, tag="vb_b", name="vb_b")
        nc.sync.dma_start(out=vb_b[:], in_=smB_d[0:1, :].to_broadcast([P, D]))
        r1m_b = persist.tile([P, D], F32, tag="r1m_b", name="r1m_b")
        nc.sync.dma_start(out=r1m_b[:], in_=smB_d[1:2, :].to_broadcast([P, D]))
        r1c_b = persist.tile([P, D], F32, tag="r1c_b", name="r1c_b")
        nc.sync.dma_start(out=r1c_b[:], in_=smB_d[2:3, :].to_broadcast([P, D]))
        r2_b = persist.tile([P, D], F32, tag="r2_b", name="r2_b")
        nc.sync.dma_start(out=r2_b[:], in_=smB_d[3:4, :].to_broadcast([P, D]))

        # int4 -> bf16 weight loads: unpack nibbles, scale per in-feature row
        def load_w4(pool, pw, dst, src_ap, scale_ap, wp):
            st = pool.tile([P, pw], U8, tag="wst", name="wst")
            lo = pool.tile([P, pw], U8, tag="wlo", name="wlo")
            hi = pool.tile([P, pw], U8, tag="whi", name="whi")
            nc.sync.dma_start(out=st[:, :wp], in_=src_ap)
            nc.vector.tensor_scalar(out=lo[:, :wp], in0=st[:, :wp], scalar1=15,
                                    scalar2=None, op0=ALU.bitwise_and)
            nc.vector.tensor_scalar(out=hi[:, :wp], in0=st[:, :wp], scalar1=4,
                                    scalar2=None, op0=ALU.logical_shift_right)
            dg = dst.rearrange("p (d t) -> p t d", t=2)
            nc.vector.tensor_scalar(out=dg[:, 0, :], in0=lo[:, :wp], scalar1=8.0,
                                    scalar2=scale_ap, op0=ALU.subtract, op1=ALU.mult)
            nc.vector.tensor_scalar(out=dg[:, 1, :], in0=hi[:, :wp], scalar1=8.0,
                                    scalar2=scale_ap, op0=ALU.subtract, op1=ALU.mult)

        # residual-stream tiles (bf16, natural layout); become x' in place.
        # Loaded as int8 + per-token scale; delta tiles d_t accumulate the
        # gamma-scaled branch sums (the device output is delta = y - x).
        xs_t = []
        for i, (n0, nn) in enumerate(NT):
            t = persist.tile([P, 2], F32, tag=f"xs{i}", name=f"xs{i}")
            nc.sync.dma_start(out=t[:nn, :], in_=xs_d[n0:n0 + nn, :])
            xs_t.append(t)
        x_t, d_t = {}, {}
        xqp = top.enter_context(tc.tile_pool(name="xq", bufs=3))
        for si, (s, src) in enumerate((("m", xm_d), ("c", xc_d))):
            for i, (n0, nn) in enumerate(NT):
                xq = xqp.tile([P, D // 2], U8, tag="xq", name="xq")
                nc.sync.dma_start(out=xq[:nn, :], in_=src[n0:n0 + nn, :])
                lo = xqp.tile([P, D // 2], U8, tag="xlo", name="xlo")
                hi = xqp.tile([P, D // 2], U8, tag="xhi", name="xhi")
                nc.vector.tensor_scalar(out=lo[:nn, :], in0=xq[:nn, :], scalar1=15,
                                        scalar2=None, op0=ALU.bitwise_and)
                nc.vector.tensor_scalar(out=hi[:nn, :], in0=xq[:nn, :], scalar1=4,
                                        scalar2=None, op0=ALU.logical_shift_right)
                t = persist.tile([P, D], BF16, tag=f"x_{s}{i}", name=f"x_{s}{i}")
                tg = t[:nn, :].rearrange("p (d t) -> p t d", t=2)
                sc_ap = xs_t[i][:nn, si:si + 1]
                nc.vector.tensor_scalar(out=tg[:, 0, :], in0=lo[:nn, :], scalar1=8.0,
                                        scalar2=sc_ap, op0=ALU.subtract, op1=ALU.mult)
                nc.vector.tensor_scalar(out=tg[:, 1, :], in0=hi[:nn, :], scalar1=8.0,
                                        scalar2=sc_ap, op0=ALU.subtract, op1=ALU.mult)
                x_t[s, i] = t
                d_t[s, i] = persist.tile([P, D], BF16, tag=f"d_{s}{i}", name=f"d_{s}{i}")

        # ---------- helpers ----------
        def layernorm_transpose(lnp, psln, s, xhatT):
            """LN over feature dim of x_t[s,*] then transpose into xhatT[j] tiles."""
            for i, (n0, nn) in enumerate(NT):
                xt = x_t[s, i]
                stats = lnp.tile([P, 3, 6], F32, tag="stats", name="stats")
                xg = xt[:nn, :].rearrange("p (g d) -> p g d", g=3)
                for g in range(3):
                    nc.vector.bn_stats(out=stats[:nn, g, :], in_=xg[:, g, :])
                mv = lnp.tile([P, 2], F32, tag="mv", name="mv")
                nc.vector.bn_aggr(out=mv[:nn], in_=stats[:nn])
                rstd = lnp.tile([P, 1], F32, tag="rstd", name="rstd")
                nc.scalar.activation(out=rstd[:nn], in_=mv[:nn, 1:2], func=AF.Sqrt,
                                     bias=eps_t[:nn], scale=1.0)
                nc.vector.reciprocal(out=rstd[:nn], in_=rstd[:nn])
                xhat = lnp.tile([P, D], BF16, tag="xhat", name="xhat")
                nc.vector.tensor_scalar(out=xhat[:nn], in0=xt[:nn, :],
                                        scalar1=mv[:nn, 0:1], scalar2=rstd[:nn],
                                        op0=ALU.subtract, op1=ALU.mult)
                for j in range(DT):
                    pst = psln.tile([P, P], BF16, tag="pst", name="pst")
                    nc.tensor.transpose(out=pst[:, :nn], in_=xhat[:nn, j * P:(j + 1) * P],
                                        identity=ident[:nn, :nn])
                    if j % 2 == 0:
                        nc.scalar.copy(out=xhatT[j][:, n0:n0 + nn], in_=pst[:, :nn])
                    else:
                        nc.vector.tensor_copy(out=xhatT[j][:, n0:n0 + nn], in_=pst[:, :nn])

        # ================= Phase A/B: LN1 + QKV =================
        # Pool lifetimes are a stack (LIFO release): ctx_io spans A/B..D and is
        # opened first; attn_io spans A/B..C and closes right after attention.
        ctx_cm = tc.tile_pool(name="ctx_io", bufs=1)
        ctx_io = ctx_cm.__enter__()
        ctxm = [ctx_io.tile([P, N], BF16, tag=f"ctxm{j}", name=f"ctxm{j}") for j in range(DT)]
        ctxc = [ctx_io.tile([P, N], BF16, tag=f"ctxc{j}", name=f"ctxc{j}") for j in range(DT)]
        attn_cm = tc.tile_pool(name="attn_io", bufs=1)
        attn_io = attn_cm.__enter__()
        qc = [attn_io.tile([P, N], BF16, tag=f"qc{h}", name=f"qc{h}") for h in range(H)]
        kc = [attn_io.tile([P, N], BF16, tag=f"kc{h}", name=f"kc{h}") for h in range(H)]
        vm = [attn_io.tile([P, H, HD + 1], BF16, tag=f"vm{i}", name=f"vm{i}") for i in range(5)]
        vc = [attn_io.tile([P, H, HD], BF16, tag=f"vc{i}", name=f"vc{i}") for i in range(5)]
        for i, (n0, nn) in enumerate(NT):
            nc.vector.memset(vm[i][:nn, :, HD:HD + 1], 1.0)

        with contextlib.ExitStack() as ab:
            wpool = ab.enter_context(tc.tile_pool(name="wqkv", bufs=1))
            wqk = [wpool.tile([P, 2 * D], BF16, tag=f"wqk{j}", name=f"wqk{j}") for j in range(DT)]
            wv = [wpool.tile([P, D], BF16, tag=f"wv{j}", name=f"wv{j}") for j in range(DT)]
            wstg1 = ab.enter_context(tc.tile_pool(name="wstg1", bufs=2))
            for j in range(DT):
                load_w4(wstg1, D, wqk[j][:], wqk_g[j * P:(j + 1) * P, :],
                        smA[:, 48 + j:49 + j], D)
                load_w4(wstg1, D, wv[j][:], wv_g[j * P:(j + 1) * P, :],
                        smA[:, 54 + j:55 + j], D // 2)

            xhatT = {s: [wpool.tile([P, N], BF16, tag=f"xhatT_{s}{j}", name=f"xhatT_{s}{j}") for j in range(DT)]
                     for s in ("m", "c")}
            lnp1 = ab.enter_context(tc.tile_pool(name="ln_ln1", bufs=3))
            psln1 = ab.enter_context(tc.tile_pool(name="psln_ln1", bufs=2, space="PSUM"))
            for s in ("m", "c"):
                layernorm_transpose(lnp1, psln1, s, xhatT[s])

            psqk = ab.enter_context(tc.tile_pool(name="psqk", bufs=3, space="PSUM"))
            sc1 = ab.enter_context(tc.tile_pool(name="sc_covqk", bufs=3))

            # --- QK GEMMs, transposed layout out [d_out, n] ---
            for s in ("m", "c"):
                for t in range(2 * DT):           # 6 q-tiles then 6 k-tiles
                    is_q = t < DT
                    for (c0, cw) in QCH:
                        ps = psqk.tile([P, 512], F32, tag="ps", name="ps")
                        for j in range(DT):
                            nc.tensor.matmul(ps[:, :cw], lhsT=wqk[j][:, t * P:(t + 1) * P],
                                             rhs=xhatT[s][j][:, c0:c0 + cw],
                                             start=(j == 0), stop=(j == DT - 1))
                        hpair = (t % DT) * 2      # heads 2*(t%6), +1
                        dst = qc if is_q else kc
                        if s == "m":
                            # mean stream: out = sc*(z + b)
                            sc = SCALE if is_q else 1.0
                            for half in range(2):
                                pr = slice(64 * half, 64 * half + 64)
                                nc.vector.tensor_scalar(
                                    out=dst[hpair + half][0:64, c0:c0 + cw],
                                    in0=ps[pr, :cw], scalar1=smA[pr, t:t + 1],
                                    scalar2=sc, op0=ALU.add, op1=ALU.mult)
                        else:
                            # cov stream: c = sqrt(elu(z + b) + 1)
                            t1 = sc1.tile([P, 512], F32, tag="t1", name="t1")
                            nc.vector.tensor_scalar_add(out=t1[:, :cw], in0=ps[:, :cw],
                                                        scalar1=smA[:, 12 + t:13 + t])
                            t2 = sc1.tile([P, 512], F32, tag="t2", name="t2")
                            nc.vector.tensor_scalar_min(out=t2[:, :cw], in0=t1[:, :cw], scalar1=0.0)
                            nc.scalar.activation(out=t2[:, :cw], in_=t2[:, :cw], func=AF.Exp)
                            nc.vector.scalar_tensor_tensor(out=t1[:, :cw], in0=t1[:, :cw],
                                                           scalar=0.0, in1=t2[:, :cw],
                                                           op0=ALU.max, op1=ALU.add)
                            for half in range(2):
                                pr = slice(64 * half, 64 * half + 64)
                                nc.scalar.activation(
                                    out=dst[hpair + half][64:128, c0:c0 + cw],
                                    in_=t1[pr, :cw], func=AF.Sqrt)

            # --- V GEMMs, natural layout out [n, d_v] ---
            for s in ("m", "c"):
                for i, (n0, nn) in enumerate(NT):
                    for c2, (v0, vw) in enumerate(VCH):
                        ps = psqk.tile([P, 512], F32, tag="ps", name="ps")
                        for j in range(DT):
                            nc.tensor.matmul(ps[:nn, :vw], lhsT=xhatT[s][j][:, n0:n0 + nn],
                                             rhs=wv[j][:, v0:v0 + vw],
                                             start=(j == 0), stop=(j == DT - 1))
                        psg = ps[:nn, :vw].rearrange("p (g d) -> p g d", g=6)
                        vbg = vb_b[:nn, v0:v0 + vw].rearrange("p (g d) -> p g d", g=6)
                        hs = slice(6 * c2, 6 * c2 + 6)
                        if s == "m":
                            nc.vector.tensor_tensor(out=vm[i][:nn, hs, 0:HD], in0=psg,
                                                    in1=vbg, op=ALU.add)
                        else:
                            t1 = sc1.tile([P, 512], F32, tag="t1", name="t1")
                            t1g = t1[:nn, :vw].rearrange("p (g d) -> p g d", g=6)
                            nc.vector.tensor_tensor(out=t1g, in0=psg, in1=vbg, op=ALU.add)
                            t2 = sc1.tile([P, 512], F32, tag="t2", name="t2")
                            nc.vector.tensor_scalar_min(out=t2[:nn, :vw], in0=t1[:nn, :vw],
                                                        scalar1=0.0)
                            nc.scalar.activation(out=t2[:nn, :vw], in_=t2[:nn, :vw], func=AF.Exp)
                            t2g = t2[:nn, :vw].rearrange("p (g d) -> p g d", g=6)
                            nc.vector.scalar_tensor_tensor(out=vc[i][:nn, hs, :], in0=t1g,
                                                           scalar=0.0, in1=t2g,
                                                           op0=ALU.max, op1=ALU.add)

        # ================= Phase C: attention =================
        with contextlib.ExitStack() as at:
            AB = at.enter_context(tc.tile_pool(name="AB", bufs=1))
            # per-head K=2 affine operands packed at 32-aligned partition slots
            # (base partition must be 0/32/64): head h -> tile h//3,
            # partitions (h%3)*32 + {0,1}. A = [colterm; ones], B = [ones; rowterm]
            N2 = N + 1   # fp32r needs even innermost extents; pad column never read
            A_pack = [AB.tile([P, N2], F32R, tag=f"A_pack{t}", name=f"A_pack{t}") for t in range(4)]
            B_pack = [AB.tile([P, N2], F32R, tag=f"B_pack{t}", name=f"B_pack{t}") for t in range(4)]

            def ab_slot(h):
                return A_pack[h // 3], B_pack[h // 3], (h % 3) * 32
            sqp = at.enter_context(tc.tile_pool(name="sqp", bufs=2))
            stg = at.enter_context(tc.tile_pool(name="stg", bufs=2))
            sigp = at.enter_context(tc.tile_pool(name="sigp", bufs=5))
            rpbp = at.enter_context(tc.tile_pool(name="rpbp", bufs=5))
            ep = at.enter_context(tc.tile_pool(name="ep", bufs=12))
            denp = at.enter_context(tc.tile_pool(name="denp", bufs=2))
            rcb = at.enter_context(tc.tile_pool(name="rcb", bufs=2))
            ps_r = at.enter_context(tc.tile_pool(name="ps_r", bufs=2, space="PSUM"))
            ps_s = at.enter_context(tc.tile_pool(name="ps_s", bufs=2, space="PSUM"))
            ps_c = at.enter_context(tc.tile_pool(name="ps_c", bufs=1, space="PSUM"))

            for h in range(H):
                # affine terms: A=[ -0.5*|w_k|^2 ; 1 ], B=[ 1 ; -0.5*|u_q|^2 ]
                A_t, B_t, sl = ab_slot(h)
                nc.sync.dma_start(out=A_t[sl + 1:sl + 2, :N], in_=ones_r[:])
                nc.vector.tensor_copy(out=B_t[sl:sl + 1, :N], in_=ones_r[:])
                sq_k = sqp.tile([P, N2], F32R, tag="sq", name="sq")
                nc.vector.tensor_tensor(out=sq_k[:, :N], in0=kc[h][:], in1=kc[h][:], op=ALU.mult)
                for (c0, cw) in QCH:
                    cwe = cw + (cw % 2)
                    pr = ps_r.tile([2, 512], F32, tag="pr", name="pr")
                    nc.tensor.matmul(pr[:, :cwe], lhsT=negh[:], rhs=sq_k[:, c0:c0 + cwe],
                                     start=True, stop=True)
                    nc.scalar.copy(out=A_t[sl:sl + 1, c0:c0 + cw], in_=pr[0:1, :cw])
                sq_q = sqp.tile([P, N2], F32R, tag="sq", name="sq")
                nc.vector.tensor_tensor(out=sq_q[:, :N], in0=qc[h][:], in1=qc[h][:], op=ALU.mult)
                rowst = stg.tile([1, N], F32R, tag="rowst", name="rowst")
                for (c0, cw) in QCH:
                    cwe = cw + (cw % 2)
                    pr = ps_r.tile([2, 512], F32, tag="pr", name="pr")
                    nc.tensor.matmul(pr[:, :cwe], lhsT=negh[:], rhs=sq_q[:, c0:c0 + cwe],
                                     start=True, stop=True)
                    nc.scalar.copy(out=rowst[0:1, c0:c0 + cw], in_=pr[0:1, :cw])
                nc.sync.dma_start(out=B_t[sl + 1:sl + 2, :N], in_=rowst[:])

                # scores + sigmoid + rpb + exp, S_T layout [k, q]
                e_h, e2_h = [], []
                for kt, (k0, kn) in enumerate(NT):
                    r0 = h * N + k0
                    rpq = rpbp.tile([P, NP2], U8, tag="rpb", name="rpb")
                    nc.sync.dma_start(out=rpq[:kn, :], in_=rpb_g[r0:r0 + kn, :])
                    rps = rpbp.tile([P, 1], F32, tag="rps", name="rps")
                    nc.sync.dma_start(out=rps[:kn, :], in_=rpbs_g[r0:r0 + kn, :])
                    rlo = rpbp.tile([P, NP2], U8, tag="rlo", name="rlo")
                    rhi = rpbp.tile([P, NP2], U8, tag="rhi", name="rhi")
                    nc.vector.tensor_scalar(out=rlo[:kn, :], in0=rpq[:kn, :], scalar1=15,
                                            scalar2=None, op0=ALU.bitwise_and)
                    nc.vector.tensor_scalar(out=rhi[:kn, :], in0=rpq[:kn, :], scalar1=4,
                                            scalar2=None, op0=ALU.logical_shift_right)
                    rpb_b = rpbp.tile([P, 2 * NP2], BF16, tag="rpbb", name="rpbb")
                    rg = rpb_b[:kn, :].rearrange("p (d t) -> p t d", t=2)
                    nc.vector.tensor_scalar_mul(out=rg[:, 0, :], in0=rlo[:kn, :],
                                                scalar1=rps[:kn, 0:1])
                    nc.vector.tensor_scalar_mul(out=rg[:, 1, :], in0=rhi[:kn, :],
                                                scalar1=rps[:kn, 0:1])
                    sig = sigp.tile([P, N], F32, tag="sig", name="sig")
                    e_t = ep.tile([P, N], BF16, tag="e", name="e")
                    e2_t = ep.tile([P, N], BF16, tag="e2", name="e2")
                    for (c0, cw) in QCH:
                        ps = ps_s.tile([P, 512], F32, tag="ps", name="ps")
                        A_t, B_t, sl = ab_slot(h)
                        kne = kn + (kn % 2)
                        cwe = cw + (cw % 2)
                        nc.tensor.matmul(ps[:kn, :cw], lhsT=kc[h][:, k0:k0 + kn],
                                         rhs=qc[h][:, c0:c0 + cw], start=True, stop=False)
                        nc.tensor.matmul(ps[:kne, :cwe], lhsT=A_t[sl:sl + 2, k0:k0 + kne],
                                         rhs=B_t[sl:sl + 2, c0:c0 + cwe], start=False, stop=True,
                                         skip_group_check=True)
                        # sigmoid(2x) = 0.5*tanh(x) + 0.5; tanh shares the ACT
                        # table set with exp.
                        nc.scalar.activation(out=sig[:kn, c0:c0 + cw], in_=ps[:kn, :cw],
                                             func=AF.Tanh, scale=1.0)
                    # e = exp(0.5*tanh + 0.5) * exp(rpb)   (rpb sent as fp8 factor)
                    nc.scalar.activation(out=sig[:kn, :], in_=sig[:kn, :], func=AF.Exp,
                                         bias=half_t[:kn], scale=0.5)
                    nc.vector.tensor_tensor(out=e_t[:kn, :], in0=sig[:kn, :],
                                            in1=rpb_b[:kn, :N], op=ALU.mult)
                    nc.gpsimd.tensor_tensor(out=e2_t[:kn, :], in0=e_t[:kn, :],
                                            in1=e_t[:kn, :], op=ALU.mult)
                    e_h.append(e_t)
                    e2_h.append(e2_t)

                # context matmuls (unnormalized) + per-chunk denominator:
                # each chunk's reciprocal/broadcast/evict chain depends only on
                # its own denominator slice, so chunks (and heads) pipeline.
                den = denp.tile([1, N], F32, tag="den", name="den")
                recip = denp.tile([1, N], F32, tag="recip", name="recip")
                rb = rcb.tile([64, N], F32, tag="rb", name="rb")
                rb2 = rcb.tile([64, N], F32, tag="rb2", name="rb2")
                jt, rr = h // 2, slice(64 * (h % 2), 64 * (h % 2) + 64)
                for ci, (c0, cw) in enumerate(QCH):
                    pm = ps_c.tile([65, 512], F32, tag=f"pcm{ci}", name=f"pcm{ci}")
                    pc2 = ps_c.tile([64, 512], F32, tag=f"pcc{ci}", name=f"pcc{ci}")
                    for kt, (k0, kn) in enumerate(NT):
                        nc.tensor.matmul(pm[:, :cw], lhsT=vm[kt][:kn, h, :],
                                         rhs=e_h[kt][:kn, c0:c0 + cw],
                                         start=(kt == 0), stop=(kt == 4))
                        nc.tensor.matmul(pc2[:, :cw], lhsT=vc[kt][:kn, h, :],
                                         rhs=e2_h[kt][:kn, c0:c0 + cw],
                                         start=(kt == 0), stop=(kt == 4))
                    nc.scalar.copy(out=den[0:1, c0:c0 + cw], in_=pm[64:65, :cw])
                    nc.vector.reciprocal(out=recip[0:1, c0:c0 + cw],
                                         in_=den[0:1, c0:c0 + cw])
                    nc.gpsimd.partition_broadcast(rb[:, c0:c0 + cw],
                                                  recip[0:1, c0:c0 + cw])
                    nc.vector.tensor_tensor(out=rb2[:, c0:c0 + cw],
                                            in0=rb[:, c0:c0 + cw],
                                            in1=rb[:, c0:c0 + cw], op=ALU.mult)
                    nc.vector.tensor_tensor(out=ctxm[jt][rr, c0:c0 + cw],
                                            in0=pm[0:64, :cw],
                                            in1=rb[:, c0:c0 + cw], op=ALU.mult)
                    nc.vector.tensor_tensor(out=ctxc[jt][rr, c0:c0 + cw],
                                            in0=pc2[0:64, :cw],
                                            in1=rb2[:, c0:c0 + cw], op=ALU.mult)

        attn_cm.__exit__(None, None, None)

        # ================= Phase D: proj + residual =================
        with contextlib.ExitStack() as pd:
            wpp = pd.enter_context(tc.tile_pool(name="wproj", bufs=1))
            wpm = [wpp.tile([P, D], BF16, tag=f"wpm{j}", name=f"wpm{j}") for j in range(DT)]
            wpc = [wpp.tile([P, D], BF16, tag=f"wpc{j}", name=f"wpc{j}") for j in range(DT)]
            wstg2 = pd.enter_context(tc.tile_pool(name="wstg2", bufs=2))
            for j in range(DT):
                load_w4(wstg2, D // 2, wpm[j][:], wpm_g[j * P:(j + 1) * P, :],
                        smA[:, 60 + j:61 + j], D // 2)
                load_w4(wstg2, D // 2, wpc[j][:], wpc_g[j * P:(j + 1) * P, :],
                        smA[:, 66 + j:67 + j], D // 2)
            psp = pd.enter_context(tc.tile_pool(name="psproj", bufs=3, space="PSUM"))
            for s, ctx_t, wp, rb_row in (("m", ctxm, wpm, r1m_b), ("c", ctxc, wpc, r1c_b)):
                for i, (n0, nn) in enumerate(NT):
                    for (v0, vw) in VCH:
                        ps = psp.tile([P, 512], F32, tag="ps", name="ps")
                        for j in range(DT):
                            nc.tensor.matmul(ps[:nn, :vw], lhsT=ctx_t[j][:, n0:n0 + nn],
                                             rhs=wp[j][:, v0:v0 + vw],
                                             start=(j == 0), stop=(j == DT - 1))
                        xt, dt = x_t[s, i], d_t[s, i]
                        # dd = gamma1*proj(ctx) + r1  (gamma folded into w-scales)
                        nc.vector.tensor_tensor(out=dt[:nn, v0:v0 + vw],
                                                in0=ps[:nn, :vw],
                                                in1=rb_row[:nn, v0:v0 + vw], op=ALU.add)
                        nc.vector.tensor_tensor(out=xt[:nn, v0:v0 + vw],
                                                in0=xt[:nn, v0:v0 + vw],
                                                in1=dt[:nn, v0:v0 + vw], op=ALU.add)

        ctx_cm.__exit__(None, None, None)

        # ================= Phase E/F: LN2 + MLP =================
        with contextlib.ExitStack() as pf:
            wfp = pf.enter_context(tc.tile_pool(name="wfc", bufs=1))
            wfc1 = [wfp.tile([P, DFF], BF16, tag=f"wfc1_{j}", name=f"wfc1_{j}") for j in range(DT)]
            wstg3 = pf.enter_context(tc.tile_pool(name="wstg3", bufs=2))
            for j in range(DT):
                load_w4(wstg3, DFF // 2, wfc1[j][:], wfc1_g[j * P:(j + 1) * P, :],
                        smA[:, 72 + j:73 + j], DFF // 2)
            wfc2 = [wfp.tile([P, D], BF16, tag=f"wfc2_{f}", name=f"wfc2_{f}") for f in range(FT)]
            for f in range(FT):
                load_w4(wstg3, DFF // 2, wfc2[f][:], wfc2_g[f * P:(f + 1) * P, :],
                        smA[:, 78 + f:79 + f], D // 2)

            xhat2T = {s: [wfp.tile([P, N], BF16, tag=f"xh2T_{s}{j}", name=f"xh2T_{s}{j}") for j in range(DT)]
                      for s in ("m", "c")}
            lnp2 = pf.enter_context(tc.tile_pool(name="ln_ln2", bufs=3))
            psln2 = pf.enter_context(tc.tile_pool(name="psln_ln2", bufs=2, space="PSUM"))
            for s in ("m", "c"):
                layernorm_transpose(lnp2, psln2, s, xhat2T[s])

            psf = pf.enter_context(tc.tile_pool(name="psfc", bufs=4, space="PSUM"))
            hp = pf.enter_context(tc.tile_pool(name="hT", bufs=1))
            outp = pf.enter_context(tc.tile_pool(name="outp", bufs=3))
            for si, s in enumerate(("m", "c")):
                # hT tiles shared between streams (tag reuse serializes via deps)
                hT = {s: [hp.tile([P, N], BF16, tag=f"hT{f}", name=f"hT{f}")
                          for f in range(FT)]}
                for f in range(FT):
                    for (c0, cw) in QCH:
                        ps = psf.tile([P, 512], F32, tag="ps", name="ps")
                        for j in range(DT):
                            nc.tensor.matmul(ps[:, :cw], lhsT=wfc1[j][:, f * P:(f + 1) * P],
                                             rhs=xhat2T[s][j][:, c0:c0 + cw],
                                             start=(j == 0), stop=(j == DT - 1))
                        nc.scalar.activation(out=hT[s][f][:, c0:c0 + cw], in_=ps[:, :cw],
                                             func=AF.Gelu, bias=smA[:, 24 + f:25 + f],
                                             scale=1.0)
                for i, (n0, nn) in enumerate(NT):
                    # delta = d_attn + gamma2*mlp(...) + r2; quantize per token
                    yt = outp.tile([P, D], F32, tag="yt", name="yt")
                    yq = outp.tile([P, D], U8, tag="yq", name="yq")
                    for (v0, vw) in VCH:
                        ps = psf.tile([P, 512], F32, tag="ps", name="ps")
                        for f in range(FT):
                            nc.tensor.matmul(ps[:nn, :vw], lhsT=hT[s][f][:, n0:n0 + nn],
                                             rhs=wfc2[f][:, v0:v0 + vw],
                                             start=(f == 0), stop=(f == FT - 1))
                        # gamma2*fc2(h) + d_attn  (gamma folded into w-scales)
                        nc.vector.tensor_tensor(out=yt[:nn, v0:v0 + vw], in0=ps[:nn, :vw],
                                                in1=d_t[s, i][:nn, v0:v0 + vw], op=ALU.add)
                        nc.vector.tensor_tensor(out=yt[:nn, v0:v0 + vw],
                                                in0=yt[:nn, v0:v0 + vw],
                                                in1=r2_b[:nn, v0:v0 + vw], op=ALU.add)
                    am = outp.tile([P, 1], F32, tag="am", name="am")
                    nc.vector.tensor_reduce(out=am[:nn], in_=yt[:nn, :], axis=AXL.X,
                                            op=ALU.max, apply_absolute_value=True)
                    sc_t = outp.tile([P, 1], F32, tag="sc", name="sc")
                    nc.vector.tensor_scalar(out=sc_t[:nn], in0=am[:nn],
                                            scalar1=1.0 / 7.0, scalar2=1e-30,
                                            op0=ALU.mult, op1=ALU.max)
                    inv = outp.tile([P, 1], F32, tag="inv", name="inv")
                    nc.vector.reciprocal(out=inv[:nn], in_=sc_t[:nn])
                    # biased nibble q+8 in [1,15], then pack pairs: lo|hi<<4
                    nc.vector.tensor_scalar(out=yq[:nn, :], in0=yt[:nn, :],
                                            scalar1=inv[:nn], scalar2=8.0,
                                            op0=ALU.mult, op1=ALU.add)
                    qg = yq[:nn, :].rearrange("p (d t) -> p t d", t=2)
                    pk = outp.tile([P, D // 2], U8, tag="pk", name="pk")
                    nc.vector.scalar_tensor_tensor(out=pk[:nn, :], in0=qg[:, 1, :],
                                                   scalar=16.0, in1=qg[:, 0, :],
                                                   op0=ALU.mult, op1=ALU.add)
                    nc.sync.dma_start(out=y_d[si * N + n0:si * N + n0 + nn, :],
                                      in_=pk[:nn, :])
                    nc.sync.dma_start(out=ysc_d[n0:n0 + nn, si:si + 1], in_=sc_t[:nn])

    nc.compile()
    return nc


def _prep_shared(inputs):
    f32 = np.float32
    g = lambda k: np.asarray(inputs[k], f32)
    qkv_w, norm1_w, norm1_b = g("qkv_w"), g("norm1_w"), g("norm1_b")
    qkv_w_eff = qkv_w * norm1_w[None, :]
    qkv_b_eff = qkv_w_eff @ norm1_b

    wqkT = np.ascontiguousarray(qkv_w_eff[:2 * D].T)
    wvT = np.ascontiguousarray(qkv_w_eff[2 * D:].T)
    qkb = qkv_b_eff[:2 * D]
    vb = qkv_b_eff[2 * D:]

    gamma1, gamma2 = g("gamma1"), g("gamma2")
    proj_w, proj_b = g("proj_w"), g("proj_b")
    cov_proj_w, cov_proj_b = g("cov_proj_w"), g("cov_proj_b")
    norm2_w, norm2_b = g("norm2_w"), g("norm2_b")
    fc1_w, fc1_b = g("fc1_w"), g("fc1_b")
    fc2_w, fc2_b = g("fc2_w"), g("fc2_b")

    fc1_w_eff = fc1_w * norm2_w[None, :]
    fc1_b_eff = fc1_b + fc1_w_eff @ norm2_b

    def pack_w(Wt):
        """Per-row symmetric int4, nibble-packed; returns (u8 [r, c/2], s [r])."""
        amax = np.abs(Wt).max(1)
        s = np.maximum(amax, 1e-30) / 7.0
        q8 = (np.rint(Wt / s[:, None]) + 8.0).astype(np.uint8)
        return q8[:, 0::2] | (q8[:, 1::2] << 4), s.astype(f32)

    p_qk, s_qk = pack_w(wqkT)
    p_v, s_v = pack_w(wvT)
    p_pm, s_pm = pack_w(np.ascontiguousarray((gamma1[:, None] * proj_w).T))
    p_pc, s_pc = pack_w(np.ascontiguousarray((gamma1[:, None] * cov_proj_w).T))
    p_f1, s_f1 = pack_w(np.ascontiguousarray(fc1_w_eff.T))
    p_f2, s_f2 = pack_w(np.ascontiguousarray((gamma2[:, None] * fc2_w).T))

    # rel_pos_bias as multiplicative exp(rpb), int4 unsigned w/ per-row scale
    NP2 = (N + 1) // 2
    rpbT = np.ascontiguousarray(
        np.asarray(inputs["rel_pos_bias"], f32)[0].transpose(0, 2, 1))
    Rp = np.zeros((RPB_PAD, 2 * NP2), f32)
    Rp[:RPB_ROWS, :N] = np.exp(rpbT.reshape(RPB_ROWS, N))
    rs = np.maximum(Rp.max(1), 1e-30) / 15.0
    Rq = np.rint(Rp / rs[:, None]).astype(np.uint8)
    rpb_pk = Rq[:, 0::2] | (Rq[:, 1::2] << 4)

    ws = np.concatenate([s_qk, s_v, s_pm, s_pc, s_f1, s_f2])   # 6912
    smA = np.concatenate([
        np.ascontiguousarray(qkb.reshape(12, P).T),
        np.ascontiguousarray(qkb.reshape(12, P).T),
        np.ascontiguousarray(fc1_b_eff.reshape(FT, P).T),
        np.ascontiguousarray(ws.reshape(54, P).T),
    ], axis=1).astype(f32)
    smB = np.stack([vb, gamma1 * proj_b, gamma1 * cov_proj_b,
                    gamma2 * fc2_b]).astype(f32)

    shared = {"smB": smB}
    full = {
        "smA_sh": smA,
        "wqk_sh": p_qk,
        "wv_sh": p_v,
        "rpb_sh": rpb_pk,
        "rpbs_sh": rs.astype(f32).reshape(RPB_PAD, 1),
        "wpm_sh": p_pm,
        "wpc_sh": p_pc,
        "wfc1_sh": p_f1,
        "wfc2_sh": p_f2,
    }
    shards = {}
    for k, v in full.items():
        ch = v.shape[0] // NC
        shards[k] = [np.ascontiguousarray(v[c * ch:(c + 1) * ch]) for c in range(NC)]
    return shared, shards


def _quant_rows4(x):
    """Per-token symmetric int4 packed 2/byte: (p [N,D/2] u8, s [N] f32)."""
    amax = np.abs(x).max(axis=1)
    s = np.maximum(amax, 1e-30) / 7.0
    q8 = (np.rint(x / s[:, None]) + 8.0).astype(np.uint8)   # [1,15]
    p = q8[:, 0::2] | (q8[:, 1::2] << 4)
    return p, s.astype(np.float32)


def _prepare(inputs):
    """Build per-core input maps (host-side prep, outside the hot path)."""
    shared, shards = _prep_shared(inputs)
    xm = np.asarray(inputs["x_mean"], np.float32)
    xc = np.asarray(inputs["x_cov"], np.float32)

    in_maps = []
    for b in range(B):
        m = dict(shared)
        for k, lst in shards.items():
            m[k] = lst[b]
        qm, sm = _quant_rows4(xm[b])
        qc_, sc = _quant_rows4(xc[b])
        m["xm"] = qm
        m["xc"] = qc_
        m["xs"] = np.stack([sm, sc], 1)
        m["_xf32"] = (xm[b], xc[b])   # host-side residual base (not a param)
        in_maps.append(m)
    return in_maps


def _make_runner(nc, n_cores):
    """Cached-jit SPMD dispatcher (mirrors bass2jax.run_bass_via_pjrt, but the
    jit is built once, and the donated zero output buffers are created
    on-device instead of being shipped from the host every call)."""
    bass2jax.install_neuronx_cc_hook()
    assert nc.dbg_addr is None

    partition_name = nc.partition_id_tensor.name if nc.partition_id_tensor else None
    in_names, out_names, out_avals, in_shapes = [], [], [], []
    for alloc in nc.m.functions[0].allocations:
        if not isinstance(alloc, mybir.MemoryLocationSet):
            continue
        name = alloc.memorylocations[0].name
        if alloc.kind == "ExternalInput":
            if name != partition_name:
                in_names.append(name)
                in_shapes.append((tuple(alloc.tensor_shape), mybir.dt.np(alloc.dtype)))
        elif alloc.kind == "ExternalOutput":
            shape = tuple(alloc.tensor_shape)
            dtype = mybir.dt.np(alloc.dtype)
            out_names.append(name)
            out_avals.append(jax.core.ShapedArray(shape, dtype))
    n_params = len(in_names)
    n_outs = len(out_names)
    param_names = list(in_names)
    in_names = in_names + out_names
    if partition_name is not None:
        in_names.append(partition_name)

    devices = jax.devices()[:n_cores]
    assert len(devices) == n_cores
    mesh = Mesh(np.asarray(devices), ("core",))
    donate = tuple(range(n_params, n_params + n_outs))

    def _body(*args):
        operands = list(args)
        if partition_name is not None:
            operands.append(bass2jax.partition_id_tensor())
        outs = bass2jax._bass_exec_p.bind(
            *operands,
            out_avals=tuple(out_avals),
            in_names=tuple(in_names),
            out_names=tuple(out_names),
            lowering_input_output_aliases=(),
            sim_require_finite=True,
            sim_require_nnan=True,
            nc=nc,
        )
        return tuple(outs)

    sm = shard_map(_body, mesh=mesh,
                   in_specs=(PartitionSpec("core"),) * (n_params + n_outs),
                   out_specs=(PartitionSpec("core"),) * n_outs,
                   check_rep=False)
    try:
        # AOT compile with bass_effect suppressed -> C++ fast-path dispatch
        example = [jax.ShapeDtypeStruct((n_cores * s[0], *s[1:]), dt)
                   for s, dt in in_shapes]
        example += [jax.ShapeDtypeStruct((n_cores * a.shape[0], *a.shape[1:]), a.dtype)
                    for a in out_avals]
        sharded = bass2jax.fast_dispatch_compile(
            lambda: jax.jit(sm, donate_argnums=donate, keep_unused=True)
            .lower(*example).compile())
    except Exception:
        sharded = jax.jit(sm, donate_argnums=donate, keep_unused=True)

    zero_shardings = tuple(NamedSharding(mesh, PartitionSpec("core"))
                           for _ in range(n_outs))

    def _zeros():
        return tuple(jnp.zeros((n_cores * a.shape[0], *a.shape[1:]), a.dtype)
                     for a in out_avals)

    zeros_jit = jax.jit(_zeros, out_shardings=zero_shardings)

    return {
        "sharded": sharded,
        "zeros_jit": zeros_jit,
        "param_names": param_names,
        "out_names": out_names,
        "out_avals": out_avals,
        "n_cores": n_cores,
    }


def _execute(in_maps):
    """One full dispatch: H2D, 8-core SPMD NEFF execution (with on-device
    AllGather of the sharded tensors), D2H, output assembly.

    The donated output buffers are recycled from the previous call (the
    kernel writes every output element, so their contents are irrelevant);
    only the first call pays for an on-device zeros launch. D2H pulls the
    8 per-device shards concurrently."""
    r = _CACHE["runner"]
    n_cores = r["n_cores"]
    per_core = [[np.asarray(m[name]) for name in r["param_names"]] for m in in_maps]
    concat_in = [
        np.concatenate([per_core[c][i] for c in range(n_cores)], axis=0)
        for i in range(len(r["param_names"]))
    ]
    don = _CACHE.pop("donate_bufs", None)
    if don is None:
        don = r["zeros_jit"]()
    out_arrs = r["sharded"](*concat_in, *don)
    _CACHE["donate_bufs"] = out_arrs

    pool = _CACHE.setdefault("d2h_pool", ThreadPoolExecutor(4 * n_cores))
    byname = dict(zip(r["out_names"], out_arrs))

    def shards_by_core(name):
        return sorted(byname[name].addressable_shards,
                      key=lambda s: s.index[0].start or 0)

    # submit ALL shard fetches first (they run 16-way concurrent), then one
    # rebuild task per core that overlaps its unpack with the fetch tail
    y_futs = [pool.submit(np.asarray, s.data) for s in shards_by_core("y")]
    s_futs = [pool.submit(np.asarray, s.data) for s in shards_by_core("ysc")]
    ym = np.empty((B, N, D), np.float32)
    yc = np.empty((B, N, D), np.float32)

    def rebuild(b):
        p_all = y_futs[b].result()                 # [2N, D/2] u8
        ysc_b = s_futs[b].result().astype(np.float32)   # [N, 2]
        xm_f, xc_f = in_maps[b]["_xf32"]
        q = np.empty((N, D), np.float32)
        for dst, xf, col in ((ym, xm_f, 0), (yc, xc_f, 1)):
            p = p_all[col * N:(col + 1) * N]
            q[:, 0::2] = (p & 15).astype(np.float32)
            q[:, 1::2] = (p >> 4).astype(np.float32)
            dst[b] = xf + (q - 8.0) * ysc_b[:, col:col + 1]

    list(pool.map(rebuild, range(B)))
    return ym, yc


def kernel(**inputs):
    if "nc" not in _CACHE:
        _CACHE["nc"] = _build_program()
        _CACHE["runner"] = _make_runner(_CACHE["nc"], NC)
    return _execute(_prepare(inputs))
